# revision 35
# baseline (speedup 1.0000x reference)
"""BLIP3o DiT block on 8 Trainium2 NeuronCores.

Strategy: data-parallel over batch (32 batches -> 4 per core), zero collectives.
On-chip layout: activations live transposed [feature, token]; matmul operands
are bf16, except the cross-attention side (eva / q2 / k2 / v2 / o2) which runs
fp8e4 with DoubleRow (2 contraction rows per PE cell, ~1.8x) -- weights are
pre-scaled by 32/64 on the host to clear the fp8 subnormal floor and the
inverse scale is folded into each PSUM drain.  Accumulation stays fp32 in
PSUM; the residual stream (hsT), modulation, rms statistics and rope tables
stay fp32.

PE-density structure (HAM clock-gate driven): every softmax-latency bubble is
filled with independent GEMM work so the PE never idles long enough to
re-throttle to K=4/8:
  phase 0   mod chunks 0..15 (quad-batched ada DMAs); rms1 + rope on vector;
            eva token-half 0 (fp8) covers the vector tail on the PE
  phase A   v1, q1, k1 (dense bf16)
  phase B   attn1 units interleaved with eva half 1 + deferred mod quads
  phase C   o1 (+rms2 issue), rms2 tail
  phase D   attn2 units interleaved with k2/q2/v2 fp8 chunk emission
  phase E   o2(t0) rides the b2 units; o2(t1) + rms3 precede the MLP;
            gate/up/down share one weight DMA across both token halves
            (catch-up re-loads only the first 4 gate/up chunks); down
            accumulates full-I in one PSUM bank; final gate/residual/store
            folded into the down drain.
Softmax denominators accumulate into the spare half of the AV PSUM bank.
Input DMAs ride the Activation HWDGE ring; weight streams ride the SP ring.
"""
import os
import sys
import numpy as np

if "/root/pylocal" not in sys.path:
    sys.path.insert(0, "/root/pylocal")  # antenv.axon_hooks shim (NTFF tracing)
try:
    import antenv
    if "/root/pylocal/antenv" not in list(antenv.__path__):
        antenv.__path__.append("/root/pylocal/antenv")
except Exception:
    pass

import concourse.bass as bass
from concourse import bacc
import concourse.mybir as mybir
from concourse.tile import TileContext
from concourse.bass_utils import run_bass_kernel_spmd

F32 = mybir.dt.float32
BF16 = mybir.dt.bfloat16
F8 = mybir.dt.float8e4
AF = mybir.ActivationFunctionType
OP = mybir.AluOpType
DR = mybir.MatmulPerfMode.DoubleRow
BF16NP = mybir.dt.np(mybir.dt.bfloat16)
F8NP = mybir.dt.np(F8)

B, S, L, H, NH, HD, I, E = 32, 256, 256, 1024, 16, 64, 4096, 4096
EPS = 1e-6
GRID = 16
NC_ = 8            # cores
BPC = B // NC_     # batches per core = 4
T = BPC * S        # tokens per core = 1024
HC = H // 128      # 8 feature chunks
EC = E // 128      # 32
IC = I // 128      # 32
NCONST = 3 * HC + HC + 48 + 1 + 192   # n1T n2T n3T eva_bT ada_bT eps ada_bT_x4
WS2 = 32.0         # host pre-scale on wq2/wk2/wv2/wo2 (fp8 range placement)
WSE = 64.0         # host pre-scale on eva_w


def _rope_tables():
    q = H // 4
    inv = 1.0 / (10000.0 ** (np.arange(0, q, dtype=np.float64)[::2] / q))  # [128]
    qd = 128
    pos_x = np.repeat(np.arange(GRID, dtype=np.float64), GRID)  # [S]
    pos_y = np.tile(np.arange(GRID, dtype=np.float64), GRID)
    fx = pos_x[:, None] * inv[None, :qd]   # [S, 128]
    fy = pos_y[:, None] * inv[None, :qd]
    t = lambda a: np.ascontiguousarray(
        np.tile(a.T.astype(np.float32), (1, BPC)))  # [128, S] -> [128, T]
    return t(np.cos(fx)), t(np.sin(fx)), t(np.cos(fy)), t(np.sin(fy))


def build_program():
    nc = bacc.Bacc()

    # ---------------- DRAM params ----------------
    d = {}
    def P(name, shape, dt, out=False):
        d[name] = nc.declare_dram_parameter(name, list(shape), dt, isOutput=out)
        return d[name]

    hsT_d = P("hsT", [H, T], F32)
    encT_d = P("encT", [E, T], F8)
    P("tembT", [H, BPC], F32)
    for w in ["wq1", "wk1", "wv1", "wo1"]:
        P(w, [H, H], BF16)
    for w in ["wq2", "wk2", "wv2", "wo2"]:
        P(w, [H, H], F8)
    P("eva_w", [E, H], F8)
    P("ada_w", [H, 6 * H], BF16)
    P("gate_w", [H, I], BF16)
    P("up_w", [H, I], BF16)
    P("down_w", [I, H], BF16)
    P("constsF", [128, NCONST], F32)   # n1T | n2T | n3T | eva_bT | ada_bT | eps
    P("ropeT", [128, 4, T], F32)       # cxt | sxt | cyt | syt
    P("ones", [128, 128], BF16)
    outT_d = P("outT", [H, T], F32, out=True)

    r3 = lambda ap: ap.rearrange("(c p) t -> p c t", p=128)

    tc_cm = TileContext(nc)
    tc = tc_cm.__enter__()

    open_pools = {}

    def pool(name, bufs=1, side="left"):
        p = tc.alloc_tile_pool(name=name, bufs=bufs, side=side)
        open_pools[name] = p
        return p

    def free(name):
        open_pools.pop(name).release()

    # long-lived small pools
    wpool = pool("wstream", bufs=4)         # tags: w8 (bf16), w8f (fp8), wq4
    sml = pool("sml", bufs=1)               # resid/gsil/outb/o2s tags
    const = pool("const", bufs=1)
    ps_proj = tc.alloc_tile_pool(name="ps_proj", bufs=2, space="PSUM")
    ps_sc = tc.alloc_tile_pool(name="ps_sc", bufs=2, space="PSUM")
    ps_av = tc.alloc_tile_pool(name="ps_av", bufs=2, space="PSUM")

    # ---------------- constants + input DMA kickoff ----------------
    tembT_sb = const.tile([128, HC, BPC], F32)
    nc.sync.dma_start(tembT_sb[:], d["tembT"].rearrange("(c p) b -> p c b", p=128))
    cst = const.tile([128, NCONST], F32)
    nc.sync.dma_start(cst[:], d["constsF"][:])
    n_sb = {"n1T": cst[:, 0:8], "n2T": cst[:, 8:16], "n3T": cst[:, 16:24],
            "eva_bT": cst[:, 24:32], "ada_bT": cst[:, 32:80]}
    eps_sb = cst[:, 80:81]
    ada_bx4 = cst[:, 81:81 + 192].rearrange("p (o b) -> p o b", b=BPC)
    ones_sb = const.tile([128, 128], BF16)
    nc.sync.dma_start(ones_sb[:], d["ones"][:])

    modT = const.tile([128, 48, BPC], F32)      # 6 splits x 8 chunks
    scale1 = const.tile([128, HC, BPC], F32)    # n1*(1+sc_msa)
    scale3 = const.tile([128, HC, BPC], F32)    # n3*(1+sc_mlp)

    # hsT / rope / enc quarters ride the Activation HWDGE ring (parallel to
    # the SP ring carrying ada/weight streams)
    p_hs = pool("p_hs")
    hsT = p_hs.tile([128, HC, T], F32)          # becomes h1T, then h2T in place
    hs_r = r3(hsT_d)
    nc.scalar.dma_start(hsT[:, 0:4], hs_r[:, 0:4])
    nc.scalar.dma_start(hsT[:, 4:8], hs_r[:, 4:8])

    p_eva = pool("p_eva")
    evaT = p_eva.tile([128, HC, T], F8)
    enc_r = r3(encT_d)  # [128, 32, T] fp8
    wev = pool("p_weva", bufs=2)
    p_wq4 = pool("p_wq4", bufs=3)   # mod-quad ada tiles; freed after attn1
    p_encA = pool("p_encA", bufs=2)  # quarters 0,1 (eva half 0); freed early
    ench = []
    for tq in range(2):   # quarters 2,3 DMA'd after phase A kickoff
        e_t = p_encA.tile([128, EC, 256], F8, tag="ench", name="ench")
        nc.scalar.dma_start(e_t[:], enc_r[:, :, tq * 256:(tq + 1) * 256])
        ench.append(e_t)

    p_rope = pool("p_rope")
    ropeT_sb = p_rope.tile([128, 4, T], F32)
    nc.scalar.dma_start(ropeT_sb[:], d["ropeT"][:])
    rope_t = {tb: ropeT_sb[:, i]
              for i, tb in enumerate(["cxt", "sxt", "cyt", "syt"])}

    ms_pools = {}

    # ---------------- rms machinery ----------------
    def rms_begin(name, x_sb, side="left"):
        rtmp = pool("rtmp_" + name, side=side)
        ps_ms = tc.alloc_tile_pool(name="ps_ms_" + name, bufs=1, space="PSUM")
        ms_pools[name] = ps_ms
        ms = [ps_ms.tile([128, 512], F32, name=f"ms_{name}_{t}")
              for t in range(2)]

        def issue(c, on_vector=False):
            sq = rtmp.tile([128, T], BF16, tag="sq", bufs=2, name=f"sq_{name}")
            if on_vector:
                nc.vector.tensor_tensor(sq[:], x_sb[:, c], x_sb[:, c], OP.mult)
            else:
                nc.scalar.activation(sq[:], x_sb[:, c], AF.Square)
            for t in range(2):
                nc.tensor.matmul(ms[t][:], ones_sb[:],
                                 sq[:, t * 512:(t + 1) * 512],
                                 start=(c == 0), stop=(c == HC - 1))

        def issue_half(c, t, on_vector=False):
            sq = rtmp.tile([128, 512], BF16, tag=f"sqh{t}", bufs=2,
                           name=f"sqh_{name}")
            src = x_sb[:, c, t * 512:(t + 1) * 512]
            if on_vector:
                nc.vector.tensor_tensor(sq[:], src, src, OP.mult)
            else:
                nc.scalar.activation(sq[:], src, AF.Square)
            nc.tensor.matmul(ms[t][:], ones_sb[:], sq[:],
                             start=(c == 0), stop=(c == HC - 1))

        def finish_half(t, consumer):
            sroot = rtmp.tile([128, 512], F32, tag="sroot", bufs=2,
                              name=f"sroot_{name}")
            nc.scalar.activation(sroot[:], ms[t][:], AF.Sqrt,
                                 bias=eps_sb, scale=1.0 / H)
            invn = rtmp.tile([128, 512], F32, tag="invn", bufs=2,
                             name=f"invn_{name}")
            nc.vector.reciprocal_approx_fast(invn[:], sroot[:])
            for c in range(HC):
                xn = rtmp.tile([128, 512], F32, tag="xn", bufs=3,
                               name=f"xn_{name}")
                nc.vector.tensor_tensor(xn[:],
                                        x_sb[:, c, t * 512:(t + 1) * 512],
                                        invn[:], OP.mult)
                consumer(c, t, xn)

        def release():
            ms_pools.pop(name).release()
            free("rtmp_" + name)

        return issue, issue_half, finish_half, release

    # x1T / rope staging (rms1 consumers write here during the mod loop)
    p_x1 = pool("p_x1", side="right")
    x1T = p_x1.tile([128, HC, T], BF16)
    p_xm = pool("p_xm")
    xm = [p_xm.tile([128, T], F32, name=f"xm{i}") for i in range(4)]

    def rms1_consumer(c, t, xn):
        dst = xm[c] if c < 4 else x1T[:, c]
        for b2 in range(2):
            b = 2 * t + b2
            nc.vector.tensor_scalar(dst[:, b * S:(b + 1) * S],
                                    xn[:, b2 * S:(b2 + 1) * S],
                                    scale1[:, c, b:b + 1],
                                    modT[:, 0 + c, b:b + 1],
                                    OP.mult, OP.add)

    r1_issue, _, r1_half, r1_release = rms_begin("r1", hsT, side="right")

    # ---------------- mod machinery (quad-batched ada DMAs) ------------------
    ada_r = r3(d["ada_w"])  # [128, 8, 6144]

    def mod_quad(o):
        """Computes modT chunks o..o+3 from one 1MB ada DMA."""
        wt = p_wq4.tile([128, HC, 512], BF16, tag="wq4", bufs=3, name="ada_t")
        # SWDGE ring: keeps the 1MB ada streams out of the SP HWDGE FIFO,
        # whose slot-gated DMAs otherwise convoy the weight prefetches
        nc.gpsimd.dma_start(wt[:], ada_r[:, :, o * 128:(o + 4) * 128])
        for pair in range(2):
            mp = ps_sc.tile([128, 2, BPC], F32, tag="sc", name="mod_ps")
            for k in range(2):
                cc = 2 * pair + k
                for f in range(HC):
                    nc.tensor.matmul(mp[:, k], wt[:, f, cc * 128:(cc + 1) * 128],
                                     stemb[:, f], start=(f == 0),
                                     stop=(f == HC - 1))
            nc.scalar.copy(modT[:, o + 2 * pair:o + 2 * pair + 2, :], mp[:])

    rp_holder = {}

    def rope_pair(pi):
        rp = rp_holder["rp"]
        (i0, i1, ct, st) = [(0, 1, "cxt", "sxt"), (2, 3, "cyt", "syt")][pi]
        a, bb = xm[i0], xm[i1]
        t1 = rp.tile([128, T], F32, tag="t1", bufs=2, name="t1")
        t2 = rp.tile([128, T], F32, tag="t2", bufs=2, name="t2")
        nc.vector.tensor_tensor(t1[:], a[:], rope_t[ct][:], OP.mult)
        nc.gpsimd.tensor_tensor(t2[:], bb[:], rope_t[st][:], OP.mult)
        nc.vector.tensor_tensor(x1T[:, i0], t1[:], t2[:], OP.subtract)
        t3 = rp.tile([128, T], F32, tag="t1", bufs=2, name="t3")
        t4 = rp.tile([128, T], F32, tag="t2", bufs=2, name="t4")
        nc.gpsimd.tensor_tensor(t3[:], a[:], rope_t[st][:], OP.mult)
        nc.vector.tensor_tensor(t4[:], bb[:], rope_t[ct][:], OP.mult)
        nc.vector.tensor_tensor(x1T[:, i1], t3[:], t4[:], OP.add)

    # ---------------- eva machinery (fp8 DoubleRow) --------------------------
    w_r_eva = r3(d["eva_w"])  # [128, 32, 1024] fp8
    eva_state = {"wq": []}

    def eva_prefetch_w(o):
        wt = wev.tile([128, EC, 128], F8, tag="weva", name="eva_w_t")
        nc.sync.dma_start(wt[:], w_r_eva[:, :, o * 128:(o + 1) * 128])
        eva_state["wq"].append(wt)

    def eva_thunk(th, o, tq2):
        i = th * HC + o

        def run():
            with nc.named_scope("eva"):
                if tq2 == 0:
                    if i + 1 < 2 * HC:
                        eva_prefetch_w((i + 1) % HC)
                    eva_state["w"] = eva_state["wq"].pop(0)
                wt = eva_state["w"]
                tq = th * 2 + tq2
                p = ps_proj.tile([128, 256], F32, tag="proj", name="eva_ps")
                for f2 in range(EC // 2):
                    nc.tensor.matmul(p[:], wt[:, 2 * f2:2 * f2 + 2, :],
                                     ench[tq][:, 2 * f2:2 * f2 + 2, :],
                                     start=(f2 == 0), stop=(f2 == EC // 2 - 1),
                                     perf_mode=DR)
                nc.vector.tensor_scalar(evaT[:, o, tq * 256:(tq + 1) * 256],
                                        p[:], 1.0 / WSE,
                                        n_sb["eva_bT"][:, o:o + 1],
                                        OP.mult, OP.add)
        return run

    eva_prefetch_w(0)
    eva_q = [eva_thunk(th, o, tq2)
             for th in range(2) for o in range(HC) for tq2 in range(2)]

    # ---------------- phase 0: mod quads 0..15 + rms1 + rope + eva th0 -------
    with nc.named_scope("mod"):
        stemb = const.tile([128, HC, BPC], BF16)
        nc.scalar.activation(stemb[:], tembT_sb[:], AF.Silu)
        for q in range(4):
            mod_quad(4 * q)
            r1_issue(2 * q, on_vector=True)
            r1_issue(2 * q + 1, on_vector=True)
        nc.vector.tensor_tensor(modT[:, 0:16], modT[:, 0:16],
                                ada_bx4[:, 0:16], OP.add)
        for c in range(HC):
            nc.vector.tensor_scalar(scale1[:, c], modT[:, 8 + c], 1.0,
                                    n_sb["n1T"][:, c:c + 1],
                                    OP.add, OP.mult)
    with nc.named_scope("rms1"):
        r1_half(0, rms1_consumer)
        r1_half(1, rms1_consumer)
        r1_release()
    with nc.named_scope("rope"):
        rp_holder["rp"] = pool("p_ropetmp")
        rope_pair(0)
        rope_pair(1)
        free("p_ropetmp")
    free("p_xm")
    # eva token-half 0 on the PE while the vector tail above drains
    for _ in range(16):
        eva_q.pop(0)()
    free("p_rope")
    free("p_encA")

    # ---------------- helpers ----------------
    def proj_chunk(name, wt, src_sb, o, consumer, KC=HC, ts=(0, 1)):
        """One output chunk o of a Y^T projection (bf16 path)."""
        for t in ts:
            p = ps_proj.tile([128, 512], F32, tag="proj", name=f"{name}_ps")
            for f in range(KC):
                nc.tensor.matmul(p[:], wt[:, f],
                                 src_sb[:, f, t * 512:(t + 1) * 512],
                                 start=(f == 0), stop=(f == KC - 1))
            consumer(o, t, p)

    def proj_chunk8(name, wt, src_sb, o, consumer, KC=HC, ts=(0, 1)):
        """fp8 DoubleRow variant: weight tile [128, KC, 128] fp8, src fp8."""
        for t in ts:
            p = ps_proj.tile([128, 512], F32, tag="proj", name=f"{name}_ps")
            for f2 in range(KC // 2):
                nc.tensor.matmul(p[:], wt[:, 2 * f2:2 * f2 + 2, :],
                                 src_sb[:, 2 * f2:2 * f2 + 2,
                                        t * 512:(t + 1) * 512],
                                 start=(f2 == 0), stop=(f2 == KC // 2 - 1),
                                 perf_mode=DR)
            consumer(o, t, p)

    def proj_T(name, w_name, src_sb, KC, consumer, OC=HC, wtag="w8"):
        w_r = r3(d[w_name])
        with nc.named_scope(name):
            for o in range(OC):
                wt = wpool.tile([128, KC, 128], BF16, tag=wtag, name=f"{name}_w")
                nc.sync.dma_start(wt[:], w_r[:, :, o * 128:(o + 1) * 128])
                proj_chunk(name, wt, src_sb, o, consumer, KC=KC)

    def copy_act(dst):
        def c(o, t, p):
            nc.scalar.copy(dst[:, o, t * 512:(t + 1) * 512], p[:])
        return c

    def copy_act_s(dst, s):
        def c(o, t, p):
            nc.scalar.activation(dst[:, o, t * 512:(t + 1) * 512], p[:],
                                 AF.Copy, scale=s)
        return c

    def vnat_chunk(scope, wt, src_sb, oh, t, dst_v):
        """V natural chunk (bf16): token chunk t (128 toks), half oh."""
        p = ps_proj.tile([128, 512], F32, tag="proj", name=f"{scope}_ps")
        KC = src_sb.shape[1]
        for f in range(KC):
            nc.tensor.matmul(p[:], src_sb[:, f, t * 128:(t + 1) * 128],
                             wt[:, f], start=(f == 0), stop=(f == KC - 1))
        nc.scalar.copy(dst_v[:, t, oh * 512:(oh + 1) * 512], p[:])

    def vnat_chunk8(scope, wt, src_sb, oh, t, dst_v, s):
        """fp8 DoubleRow V natural chunk; drain rescales by s."""
        p = ps_proj.tile([128, 512], F32, tag="proj", name=f"{scope}_ps")
        KC = src_sb.shape[1]
        for f2 in range(KC // 2):
            nc.tensor.matmul(p[:],
                             src_sb[:, 2 * f2:2 * f2 + 2, t * 128:(t + 1) * 128],
                             wt[:, 2 * f2:2 * f2 + 2, :],
                             start=(f2 == 0), stop=(f2 == KC // 2 - 1),
                             perf_mode=DR)
        nc.scalar.activation(dst_v[:, t, oh * 512:(oh + 1) * 512], p[:],
                             AF.Copy, scale=s)

    def vnat(w_name, src_sb, dst_v, scope, side="left"):
        w_r = r3(d[w_name])  # [128, HC, H]
        wv = pool("wv_" + scope, bufs=2, side=side)
        with nc.named_scope(scope):
            for oh in range(2):
                wt = wv.tile([128, HC, 512], BF16, tag="wvnat", name=f"{scope}_w")
                nc.sync.dma_start(wt[:], w_r[:, :, oh * 512:(oh + 1) * 512])
                for t in range(2 * BPC):
                    vnat_chunk(scope, wt, src_sb, oh, t, dst_v)
        free("wv_" + scope)

    # ---------------- attention unit machinery ----------------
    at_store = {}

    def attn_A(attnp, qt_sb, kt_sb, b, hc):
        at = []
        for ho in range(2):
            sc_ps = ps_sc.tile([128, 2, S], F32, tag="sc", name="sc_ps")
            for kc in range(2):
                nc.tensor.matmul(
                    sc_ps[:, kc],
                    kt_sb[ho * 64:(ho + 1) * 64, hc,
                          b * S + kc * 128: b * S + (kc + 1) * 128],
                    qt_sb[ho * 64:(ho + 1) * 64, hc, b * S:(b + 1) * S],
                    start=True, stop=True)
            a = attnp.tile([128, 2, S], BF16, tag=f"attn{ho}", bufs=2,
                           name="attn_sb")
            nc.scalar.activation(a[:], sc_ps[:], AF.Exp,
                                 scale=float(HD) ** -0.5)
            at.append(a)
        at_store[(b, hc)] = at

    def attn_B(attnp, vp_sb, out_sb, b, hc):
        at = at_store.pop((b, hc))
        for ho in range(2):
            h = 2 * hc + ho
            av = ps_av.tile([64, 512], F32, tag="av", name="av_ps")
            for kc in range(2):
                nc.tensor.matmul(av[:, 0:256],
                                 vp_sb[:, b * 2 + kc, h * 64:(h + 1) * 64],
                                 at[ho][:, kc],
                                 start=(kc == 0), stop=(kc == 1))
            for kc in range(2):
                nc.tensor.matmul(av[:, 256:512], ones_sb[:, 0:64],
                                 at[ho][:, kc],
                                 start=(kc == 0), stop=(kc == 1))
            inv = attnp.tile([64, S], F32, tag="inv", name="inv_sb")
            nc.vector.reciprocal_approx_fast(inv[:], av[:, 256:512])
            nc.vector.tensor_tensor(
                out_sb[ho * 64:(ho + 1) * 64, hc, b * S:(b + 1) * S],
                av[:, 0:256], inv[:], OP.mult)

    # ---------------- phase A: V1, Q1, K1 (dense bf16 GEMMs) -----------------
    p_vp = pool("p_vp")
    vp = p_vp.tile([128, 2 * BPC, NH * 64], BF16)
    p_encB = pool("p_encB", bufs=2)
    vnat("wv1", x1T, vp, "v1")
    # enc quarters 2,3 (needed by eva half 1 inside the attn1 window)
    for tq in range(2, 4):
        e_t = p_encB.tile([128, EC, 256], F8, tag="ench", name="ench")
        nc.scalar.dma_start(e_t[:], enc_r[:, :, tq * 256:(tq + 1) * 256])
        ench.append(e_t)

    p_qt = pool("p_qt"); qt = p_qt.tile([128, HC, T], BF16)
    p_kt = pool("p_kt"); kt = p_kt.tile([128, HC, T], BF16)
    proj_T("q1", "wq1", x1T, HC, copy_act(qt))
    proj_T("k1", "wk1", x1T, HC, copy_act(kt))
    free("p_x1")

    # ---------------- phase B: attn1 || eva th1 || mod quads 16..47 ----------
    p_ao = pool("p_ao", side="right")
    attnout = p_ao.tile([128, HC, T], BF16)
    attnp1 = pool("attnp_attn1", bufs=3, side="right")

    def mod_thunk(o):
        def run():
            with nc.named_scope("mod"):
                mod_quad(o)
        return run

    def mod_final():
        with nc.named_scope("mod"):
            nc.vector.tensor_tensor(modT[:, 16:48], modT[:, 16:48],
                                    ada_bx4[:, 16:48], OP.add)
            for c in range(HC):
                nc.vector.tensor_scalar(scale3[:, c], modT[:, 32 + c], 1.0,
                                        n_sb["n3T"][:, c:c + 1],
                                        OP.add, OP.mult)

    # filler queue: 2 eva chunks then 1 mod quad, repeating (24 items for the
    # 24 filler slots among 32 units -- every 4th unit runs bare)
    fillers = []
    mq = [mod_thunk(o) for o in range(16, 48, 4)]
    for g in range(8):
        fillers.append(eva_q.pop(0))
        fillers.append(eva_q.pop(0))
        fillers.append(mq.pop(0))

    with nc.named_scope("attn1"):
        # batch-pair interleave: the partner unit's score MMs + fillers hide
        # each unit's exp latency
        for bp in (0, 2):
            for hc in range(NH // 2):
                attn_A(attnp1, qt, kt, bp, hc)
                attn_A(attnp1, qt, kt, bp + 1, hc)
                if fillers:
                    fillers.pop(0)()
                if hc % 2 == 0 and fillers:
                    fillers.pop(0)()
                attn_B(attnp1, vp, attnout, bp, hc)
                attn_B(attnp1, vp, attnout, bp + 1, hc)
        while fillers:
            fillers.pop(0)()
        mod_final()
    free("attnp_attn1")
    free("p_kt"); free("p_qt"); free("p_encB"); free("p_vp")
    free("p_wq4")
    free("p_weva")

    # ---------------- phase C: o1 (+rms2), rms2 tail -------------------------
    p_r2 = pool("p_r2")
    rms2T = p_r2.tile([128, HC, T], F8)
    r2_issue, _, r2_half, r2_release = rms_begin("r2", hsT, side="right")

    # prefetch cross-attention weights while o1 runs
    wv2 = pool("wv_v2", bufs=2)
    w_r_v2 = r3(d["wv2"])
    wv2_t = []
    for oh in range(2):
        wt = wv2.tile([128, HC, 512], F8, tag="wvnat", name="v2_w")
        nc.sync.dma_start(wt[:], w_r_v2[:, :, oh * 512:(oh + 1) * 512])
        wv2_t.append(wt)
    w_r_k2 = r3(d["wk2"])
    w_r_q2 = r3(d["wq2"])
    kq_tiles = []

    def kq_prefetch(hc):
        wtk = wpool.tile([128, HC, 128], F8, tag="w8f", name="k2_w")
        nc.sync.dma_start(wtk[:], w_r_k2[:, :, hc * 128:(hc + 1) * 128])
        wtq = wpool.tile([128, HC, 128], F8, tag="w8f", name="q2_w")
        nc.sync.dma_start(wtq[:], w_r_q2[:, :, hc * 128:(hc + 1) * 128])
        kq_tiles.append((wtk, wtq))

    kq_prefetch(0)

    def resid_gated(g_split, rms_issue=None):
        def c(o, t, p):
            tg = sml.tile([128, 512], F32, tag="resid", name="resid_t")
            for b2 in range(2):
                b = t * 2 + b2
                nc.vector.tensor_scalar(tg[:, b2 * S:(b2 + 1) * S],
                                        p[:, b2 * S:(b2 + 1) * S],
                                        modT[:, g_split * 8 + o, b:b + 1],
                                        None, OP.mult)
            nc.vector.tensor_tensor(hsT[:, o, t * 512:(t + 1) * 512],
                                    hsT[:, o, t * 512:(t + 1) * 512],
                                    tg[:], OP.add)
            if t == 1 and rms_issue is not None:
                rms_issue(o)
        return c

    proj_T("o1", "wo1", attnout, HC, resid_gated(2, r2_issue))

    def rms2_consumer(c, t, xn):
        nc.vector.tensor_scalar(rms2T[:, c, t * 512:(t + 1) * 512], xn[:],
                                n_sb["n2T"][:, c:c + 1], None, OP.mult)

    with nc.named_scope("rms2"):
        r2_half(0, rms2_consumer)   # hsT now holds h1
        r2_half(1, rms2_consumer)
    r2_release()
    free("p_ao")

    # ---------------- phase D: cross attention || k2/q2/v2 (fp8) -------------
    p_ao2 = pool("p_ao2", side="right")
    attn2out = p_ao2.tile([128, HC, T], F8)
    attnp2 = pool("attnp_attn2", bufs=3, side="right")
    p_k2 = pool("p_k2", side="right"); k2t = p_k2.tile([128, HC, T], BF16)
    p_q2 = pool("p_q2", side="right"); q2t = p_q2.tile([128, HC, T], BF16)
    p_v2 = pool("p_v2", side="right")
    vp2 = p_v2.tile([128, 2 * BPC, NH * 64], BF16)

    with nc.named_scope("attn2"):
        # V for b0+b1 (token chunks 0..3, both halves) up front
        for t in range(4):
            for oh in range(2):
                vnat_chunk8("v2", wv2_t[oh], evaT, oh, t, vp2, 1.0 / WS2)
        # b0/b1 unit pairs pipelined with k2/q2 chunk emission; V chunks for
        # b2/b3 ride along as extra filler
        vfill = [(t, oh) for t in range(4, 2 * BPC) for oh in range(2)]
        for hc in range(HC):
            if hc + 1 < HC:
                kq_prefetch(hc + 1)
            wtk, wtq = kq_tiles.pop(0)
            proj_chunk8("k2", wtk, evaT, hc, copy_act_s(k2t, 1.0 / WS2))
            proj_chunk8("q2", wtq, rms2T, hc, copy_act_s(q2t, 1.0 / WS2))
            if hc >= 1:
                attn_B(attnp2, vp2, attn2out, 0, hc - 1)
                attn_B(attnp2, vp2, attn2out, 1, hc - 1)
            attn_A(attnp2, q2t, k2t, 0, hc)
            attn_A(attnp2, q2t, k2t, 1, hc)
            if vfill:
                t, oh = vfill.pop(0)
                vnat_chunk8("v2", wv2_t[oh], evaT, oh, t, vp2, 1.0 / WS2)
        attn_B(attnp2, vp2, attn2out, 0, HC - 1)
        attn_B(attnp2, vp2, attn2out, 1, HC - 1)
    free("wv_v2")
    free("p_r2")
    free("p_eva")

    # ---------------- phase E: o2/rms3 by token halves, then MLP -------------
    p_y = pool("p_y")
    yT = p_y.tile([128, HC, T], BF16)
    _, r3_issue_h, r3_half, _ = rms_begin("r3", hsT, side="left")

    wo2p = pool("p_wo2", bufs=1)
    wo2_t = wo2p.tile([128, HC, HC, 128], F8)
    w_r_o2 = r3(d["wo2"])
    for o in range(HC):
        nc.sync.dma_start(wo2_t[:, o], w_r_o2[:, :, o * 128:(o + 1) * 128])

    def resid_plain(o, t, p):
        tmp = sml.tile([128, 512], F32, tag="o2s", bufs=2, name="o2s")
        nc.scalar.activation(tmp[:], p[:], AF.Copy, scale=1.0 / WS2)
        nc.vector.tensor_tensor(hsT[:, o, t * 512:(t + 1) * 512],
                                hsT[:, o, t * 512:(t + 1) * 512],
                                tmp[:], OP.add)
        r3_issue_h(o, t)

    def rms3_consumer(c, t, xn):
        for b2 in range(2):
            b = 2 * t + b2
            nc.vector.tensor_scalar(yT[:, c, b * S:(b + 1) * S],
                                    xn[:, b2 * S:(b2 + 1) * S],
                                    scale3[:, c, b:b + 1],
                                    modT[:, 24 + c, b:b + 1],
                                    OP.mult, OP.add)

    with nc.named_scope("attn2"):
        # b2/b3 unit pairs with o2(t0) chunks as filler
        for hc in range(HC):
            attn_A(attnp2, q2t, k2t, 2, hc)
            attn_A(attnp2, q2t, k2t, 3, hc)
            with nc.named_scope("o2"):
                proj_chunk8("o2", wo2_t[:, hc], attn2out, hc, resid_plain,
                            ts=(0,))
            attn_B(attnp2, vp2, attn2out, 2, hc)
            attn_B(attnp2, vp2, attn2out, 3, hc)
    # q2/k2/v2 and the exp tiles are dead once the b3 units are emitted;
    # free (LIFO) before the MLP reserves its 64KB/partition of mlpT slots.
    free("p_v2"); free("p_q2"); free("p_k2"); free("attnp_attn2")

    # rms3 half0 vector tail hides under the o2(t1) PE stream
    with nc.named_scope("rms3"):
        r3_half(0, rms3_consumer)
    with nc.named_scope("o2"):
        for o in range(HC):
            proj_chunk8("o2", wo2_t[:, o], attn2out, o, resid_plain, ts=(1,))
    free("p_ao2")
    with nc.named_scope("rms3"):
        r3_half(1, rms3_consumer)

    # ---------------- MLP: gate/up/down, weights shared across halves --------
    gate_r = r3(d["gate_w"])  # [128, 8, 4096]
    up_r = r3(d["up_w"])
    down_r = r3(d["down_w"])  # [128, 32, 1024]
    out_r = r3(outT_d)
    p_mlp = pool("p_mlp")
    wmlp = pool("p_wmlp", bufs=5)
    wdn = pool("p_wdown", bufs=2)
    mlpT = [p_mlp.tile([128, IC, 512], BF16, tag="mlp", bufs=2,
                       name=f"mlpT{t}") for t in range(2)]

    def gu_mm(wg, wu, oc, ts):
        for t in ts:
            pg = ps_proj.tile([128, 512], F32, tag="proj", name="g_ps")
            for f in range(HC):
                nc.tensor.matmul(pg[:], wg[:, f],
                                 yT[:, f, t * 512:(t + 1) * 512],
                                 start=(f == 0), stop=(f == HC - 1))
            pu = ps_proj.tile([128, 512], F32, tag="proj", name="u_ps")
            for f in range(HC):
                nc.tensor.matmul(pu[:], wu[:, f],
                                 yT[:, f, t * 512:(t + 1) * 512],
                                 start=(f == 0), stop=(f == HC - 1))
            gs = sml.tile([128, 512], BF16, tag="gsil", name="gsil")
            nc.scalar.activation(gs[:], pg[:], AF.Silu)
            nc.vector.tensor_tensor(mlpT[t][:, oc], gs[:], pu[:], OP.mult)

    def gu_chunk(oc, ts):
        wg = wmlp.tile([128, HC, 128], BF16, tag="w8", name="gate_w_t")
        nc.sync.dma_start(wg[:], gate_r[:, :, oc * 128:(oc + 1) * 128])
        wu = wmlp.tile([128, HC, 128], BF16, tag="w8", name="up_w_t")
        nc.sync.dma_start(wu[:], up_r[:, :, oc * 128:(oc + 1) * 128])
        gu_mm(wg, wu, oc, ts)

    with nc.named_scope("gateup0"):
        for oc in range(4):
            gu_chunk(oc, (0,))
    with nc.named_scope("gateup1"):
        for oc in range(4, IC):
            gu_chunk(oc, (0, 1))
        for oc in range(4):
            gu_chunk(oc, (1,))

    with nc.named_scope("down0"):
        for o in range(HC):
            wt = wdn.tile([128, IC, 128], BF16, tag="wdown", name="down_w_t")
            nc.sync.dma_start(wt[:], down_r[:, :, o * 128:(o + 1) * 128])
            for t in range(2):
                p = ps_proj.tile([128, 512], F32, tag="proj", name="d_ps")
                for f in range(IC):
                    nc.tensor.matmul(p[:], wt[:, f], mlpT[t][:, f],
                                     start=(f == 0), stop=(f == IC - 1))
                sl = slice(t * 512, (t + 1) * 512)
                ob = sml.tile([128, 512], F32, tag="outb", bufs=2, name="outb")
                for b2 in range(2):
                    b = 2 * t + b2
                    nc.vector.tensor_scalar(ob[:, b2 * S:(b2 + 1) * S],
                                            p[:, b2 * S:(b2 + 1) * S],
                                            modT[:, 40 + o, b:b + 1],
                                            None, OP.mult)
                nc.vector.tensor_tensor(ob[:], ob[:], hsT[:, o, sl], OP.add)
                nc.sync.dma_start(out_r[:, o, sl], ob[:])

    for nm in reversed(list(open_pools)):
        free(nm)
    for p in list(ms_pools.values()):
        p.release()
    ps_av.release(); ps_sc.release(); ps_proj.release()
    tc_cm.__exit__(None, None, None)
    nc.compile()
    return nc


_CACHE = {}


def _get_program():
    if "nc" not in _CACHE:
        _CACHE["nc"] = build_program()
    return _CACHE["nc"]


def kernel(hidden_states, encoder_hidden_states, timestep_emb,
           wq1, wk1, wv1, wo1, wq2, wk2, wv2, wo2,
           eva_w, eva_b, ada_w, ada_b, gate_w, up_w, down_w, n1, n2, n3,
           _trace=False):
    nc = _get_program()
    f32 = lambda a: np.ascontiguousarray(np.asarray(a), dtype=np.float32)
    bf = lambda a: np.ascontiguousarray(np.asarray(a), dtype=np.float32).astype(BF16NP)
    f8s = lambda a, s: np.ascontiguousarray(
        np.asarray(a, dtype=np.float32) * s).astype(F8NP)

    cxt, sxt, cyt, syt = _rope_tables()
    ropeT = np.ascontiguousarray(np.stack([cxt, sxt, cyt, syt], axis=1))
    colchunks = lambda v, n: np.asarray(v, np.float32).reshape(n, 128).T
    ada_bT = colchunks(ada_b, 48)
    constsF = np.concatenate([
        colchunks(n1, HC), colchunks(n2, HC), colchunks(n3, HC),
        colchunks(eva_b, HC), ada_bT,
        np.full((128, 1), EPS, np.float32),
        np.repeat(ada_bT, 4, axis=1)], axis=1)
    shared = dict(
        wq1=bf(wq1), wk1=bf(wk1), wv1=bf(wv1), wo1=bf(wo1),
        wq2=f8s(wq2, WS2), wk2=f8s(wk2, WS2), wv2=f8s(wv2, WS2),
        wo2=f8s(wo2, WS2),
        eva_w=f8s(eva_w, WSE), ada_w=bf(ada_w), gate_w=bf(gate_w),
        up_w=bf(up_w), down_w=bf(down_w),
        constsF=np.ascontiguousarray(constsF),
        ropeT=ropeT,
        ones=np.ones((128, 128), BF16NP),
    )
    hs = f32(hidden_states)
    enc = f32(encoder_hidden_states)
    temb = f32(timestep_emb)

    in_maps = []
    for c in range(NC_):
        sl = slice(c * BPC, (c + 1) * BPC)
        m = dict(shared)
        m["hsT"] = np.ascontiguousarray(hs[sl].transpose(2, 0, 1).reshape(H, T))
        m["encT"] = np.ascontiguousarray(
            enc[sl].transpose(2, 0, 1).reshape(E, T)).astype(F8NP)
        m["tembT"] = np.ascontiguousarray(temb[sl].T)
        in_maps.append(m)

    res = run_bass_kernel_spmd(nc, in_maps, core_ids=list(range(NC_)),
                               trace=_trace)
    out = np.empty((B, S, H), np.float32)
    for c in range(NC_):
        o = res.results[c]["outT"]  # [H, T]
        out[c * BPC:(c + 1) * BPC] = np.ascontiguousarray(o.T).reshape(BPC, S, H)
    if _trace:
        kernel.last_results = res
    return out


# revision 51
# speedup vs baseline: 1.0320x; 1.0320x over previous
"""BLIP3o DiT block on 8 Trainium2 NeuronCores.

Strategy: data-parallel over batch (32 batches -> 4 per core), zero collectives.
On-chip layout: activations live transposed [feature, token]; matmul operands
are bf16, except the cross-attention side (eva / q2 / k2 / v2 / o2) which runs
fp8e4 with DoubleRow (2 contraction rows per PE cell, ~1.8x) -- weights are
pre-scaled by 32/64 on the host to clear the fp8 subnormal floor and the
inverse scale is folded into each PSUM drain.  Accumulation stays fp32 in
PSUM; the residual stream (hsT), modulation, rms statistics and rope tables
stay fp32.

PE-density structure (HAM clock-gate driven): every softmax-latency bubble is
filled with independent GEMM work so the PE never idles long enough to
re-throttle to K=4/8:
  phase 0   mod chunks 0..15 (quad-batched ada DMAs); rms1 + rope on vector;
            eva token-half 0 (fp8) covers the vector tail on the PE
  phase A   v1, q1, k1 (dense bf16)
  phase B   attn1 units interleaved with eva half 1 + deferred mod quads
  phase C   o1 (+rms2 issue), rms2 tail
  phase D   attn2 units interleaved with k2/q2/v2 fp8 chunk emission
  phase E   o2(t0) rides the b2 units; o2(t1) + rms3 precede the MLP;
            gate/up/down share one weight DMA across both token halves
            (catch-up re-loads only the first 4 gate/up chunks); down
            accumulates full-I in one PSUM bank; final gate/residual/store
            folded into the down drain.
Softmax denominators accumulate into the spare half of the AV PSUM bank.
Input DMAs ride the Activation HWDGE ring; weight streams ride the SP ring.
"""
import os
import sys
import numpy as np

if "/root/pylocal" not in sys.path:
    sys.path.insert(0, "/root/pylocal")  # antenv.axon_hooks shim (NTFF tracing)
try:
    import antenv
    if "/root/pylocal/antenv" not in list(antenv.__path__):
        antenv.__path__.append("/root/pylocal/antenv")
except Exception:
    pass

import concourse.bass as bass
from concourse import bacc
import concourse.mybir as mybir
from concourse.tile import TileContext
from concourse.bass_utils import run_bass_kernel_spmd

F32 = mybir.dt.float32
BF16 = mybir.dt.bfloat16
F8 = mybir.dt.float8e4
AF = mybir.ActivationFunctionType
OP = mybir.AluOpType
DR = mybir.MatmulPerfMode.DoubleRow
BF16NP = mybir.dt.np(mybir.dt.bfloat16)
F8NP = mybir.dt.np(F8)

B, S, L, H, NH, HD, I, E = 32, 256, 256, 1024, 16, 64, 4096, 4096
EPS = 1e-6
GRID = 16
NC_ = 8            # cores
BPC = B // NC_     # batches per core = 4
T = BPC * S        # tokens per core = 1024
HC = H // 128      # 8 feature chunks
EC = E // 128      # 32
IC = I // 128      # 32
NCONST = 3 * HC + HC + 48 + 1 + 192   # n1T n2T n3T eva_bT ada_bT eps ada_bT_x4
WS2 = 32.0         # host pre-scale on wq2/wk2/wv2/wo2 (fp8 range placement)
WSE = 64.0         # host pre-scale on eva_w


def _rope_tables():
    q = H // 4
    inv = 1.0 / (10000.0 ** (np.arange(0, q, dtype=np.float64)[::2] / q))  # [128]
    qd = 128
    pos_x = np.repeat(np.arange(GRID, dtype=np.float64), GRID)  # [S]
    pos_y = np.tile(np.arange(GRID, dtype=np.float64), GRID)
    fx = pos_x[:, None] * inv[None, :qd]   # [S, 128]
    fy = pos_y[:, None] * inv[None, :qd]
    t = lambda a: np.ascontiguousarray(
        np.tile(a.T.astype(np.float32), (1, BPC)))  # [128, S] -> [128, T]
    return t(np.cos(fx)), t(np.sin(fx)), t(np.cos(fy)), t(np.sin(fy))


def build_program():
    nc = bacc.Bacc()

    # ---------------- DRAM params ----------------
    d = {}
    def P(name, shape, dt, out=False):
        d[name] = nc.declare_dram_parameter(name, list(shape), dt, isOutput=out)
        return d[name]

    # weights are host-packed tile-major [OC, 128, KC, cols] so every weight
    # DMA is one fully-contiguous read (the (c p) rearrange view otherwise
    # yields 128-256B DRAM runs and ~50-150GB/s effective DMA)
    hsT_d = P("hsT", [H, T], F32)
    P("encT", [4, 128, EC, 256], F8)
    P("tembT", [H, BPC], F32)
    for w in ["wq1", "wk1", "wo1"]:
        P(w, [HC, 128, HC, 128], BF16)
    P("wv1", [2, 128, HC, 512], BF16)
    for w in ["wq2", "wk2", "wo2"]:
        P(w, [HC, 128, HC, 128], F8)
    P("wv2", [2, 128, HC, 512], F8)
    P("eva_w", [HC, 128, EC, 128], F8)
    P("ada_w", [12, 128, HC, 512], BF16)
    P("gate_w", [IC, 128, HC, 128], BF16)
    P("up_w", [IC, 128, HC, 128], BF16)
    P("down_w", [HC, 128, IC, 128], BF16)
    P("constsF", [128, NCONST], F32)   # n1T | n2T | n3T | eva_bT | ada_bT | eps
    P("ropeT", [128, 4, T], F32)       # cxt | sxt | cyt | syt
    P("ones", [128, 128], BF16)
    outT_d = P("outT", [H, T], F32, out=True)

    r3 = lambda ap: ap.rearrange("(c p) t -> p c t", p=128)

    tc_cm = TileContext(nc)
    tc = tc_cm.__enter__()

    open_pools = {}

    def pool(name, bufs=1, side="left"):
        p = tc.alloc_tile_pool(name=name, bufs=bufs, side=side)
        open_pools[name] = p
        return p

    def free(name):
        open_pools.pop(name).release()

    # long-lived small pools
    wpool = pool("wstream", bufs=4)         # tags: w8 (bf16), w8f (fp8), wq4
    sml = pool("sml", bufs=1)               # resid/gsil/outb/o2s tags
    const = pool("const", bufs=1)
    ps_proj = tc.alloc_tile_pool(name="ps_proj", bufs=2, space="PSUM")
    ps_sc = tc.alloc_tile_pool(name="ps_sc", bufs=2, space="PSUM")
    ps_av = tc.alloc_tile_pool(name="ps_av", bufs=2, space="PSUM")

    # ---------------- constants + input DMA kickoff ----------------
    tembT_sb = const.tile([128, HC, BPC], F32)
    nc.sync.dma_start(tembT_sb[:], d["tembT"].rearrange("(c p) b -> p c b", p=128))
    cst = const.tile([128, NCONST], F32)
    nc.sync.dma_start(cst[:], d["constsF"][:])
    n_sb = {"n1T": cst[:, 0:8], "n2T": cst[:, 8:16], "n3T": cst[:, 16:24],
            "eva_bT": cst[:, 24:32], "ada_bT": cst[:, 32:80]}
    eps_sb = cst[:, 80:81]
    ada_bx4 = cst[:, 81:81 + 192].rearrange("p (o b) -> p o b", b=BPC)
    ones_sb = const.tile([128, 128], BF16)
    nc.sync.dma_start(ones_sb[:], d["ones"][:])

    modT = const.tile([128, 48, BPC], F32)      # 6 splits x 8 chunks
    scale1 = const.tile([128, HC, BPC], F32)    # n1*(1+sc_msa)
    scale3 = const.tile([128, HC, BPC], F32)    # n3*(1+sc_mlp)

    # hsT / rope / enc quarters ride the Activation HWDGE ring (parallel to
    # the SP ring carrying ada/weight streams)
    p_hs = pool("p_hs")
    hsT = p_hs.tile([128, HC, T], F32)          # becomes h1T, then h2T in place
    hs_r = r3(hsT_d)
    nc.scalar.dma_start(hsT[:, 0:4], hs_r[:, 0:4])
    nc.scalar.dma_start(hsT[:, 4:8], hs_r[:, 4:8])

    p_eva = pool("p_eva")
    evaT = p_eva.tile([128, HC, T], F8)
    wev = pool("p_weva", bufs=2)
    p_wq4 = pool("p_wq4", bufs=3)   # mod-quad ada tiles; freed after attn1
    p_encA = pool("p_encA", bufs=2)  # quarters 0,1 (eva half 0); freed early
    ench = []
    for tq in range(2):   # quarters 2,3 DMA'd after phase A kickoff
        e_t = p_encA.tile([128, EC, 256], F8, tag="ench", name="ench")
        nc.scalar.dma_start(e_t[:], d["encT"][tq])
        ench.append(e_t)

    p_rope = pool("p_rope")
    ropeT_sb = p_rope.tile([128, 4, T], F32)
    nc.scalar.dma_start(ropeT_sb[:], d["ropeT"][:])
    rope_t = {tb: ropeT_sb[:, i]
              for i, tb in enumerate(["cxt", "sxt", "cyt", "syt"])}

    ms_pools = {}

    # ---------------- rms machinery ----------------
    def rms_begin(name, x_sb, side="left"):
        rtmp = pool("rtmp_" + name, side=side)
        ps_ms = tc.alloc_tile_pool(name="ps_ms_" + name, bufs=1, space="PSUM")
        ms_pools[name] = ps_ms
        ms = [ps_ms.tile([128, 512], F32, name=f"ms_{name}_{t}")
              for t in range(2)]

        def issue(c, on_vector=False):
            sq = rtmp.tile([128, T], BF16, tag="sq", bufs=2, name=f"sq_{name}")
            if on_vector:
                nc.vector.tensor_tensor(sq[:], x_sb[:, c], x_sb[:, c], OP.mult)
            else:
                nc.scalar.activation(sq[:], x_sb[:, c], AF.Square)
            for t in range(2):
                nc.tensor.matmul(ms[t][:], ones_sb[:],
                                 sq[:, t * 512:(t + 1) * 512],
                                 start=(c == 0), stop=(c == HC - 1))

        def issue_half(c, t, on_vector=False):
            sq = rtmp.tile([128, 512], BF16, tag=f"sqh{t}", bufs=2,
                           name=f"sqh_{name}")
            src = x_sb[:, c, t * 512:(t + 1) * 512]
            if on_vector:
                nc.vector.tensor_tensor(sq[:], src, src, OP.mult)
            else:
                nc.scalar.activation(sq[:], src, AF.Square)
            nc.tensor.matmul(ms[t][:], ones_sb[:], sq[:],
                             start=(c == 0), stop=(c == HC - 1))

        def finish_half(t, consumer):
            sroot = rtmp.tile([128, 512], F32, tag="sroot", bufs=2,
                              name=f"sroot_{name}")
            nc.scalar.activation(sroot[:], ms[t][:], AF.Sqrt,
                                 bias=eps_sb, scale=1.0 / H)
            invn = rtmp.tile([128, 512], F32, tag="invn", bufs=2,
                             name=f"invn_{name}")
            nc.vector.reciprocal_approx_fast(invn[:], sroot[:])
            for c in range(HC):
                xn = rtmp.tile([128, 512], F32, tag="xn", bufs=3,
                               name=f"xn_{name}")
                nc.vector.tensor_tensor(xn[:],
                                        x_sb[:, c, t * 512:(t + 1) * 512],
                                        invn[:], OP.mult)
                consumer(c, t, xn)

        def release():
            ms_pools.pop(name).release()
            free("rtmp_" + name)

        return issue, issue_half, finish_half, release

    # x1T / rope staging (rms1 consumers write here during the mod loop)
    p_x1 = pool("p_x1", side="right")
    x1T = p_x1.tile([128, HC, T], BF16)
    p_xm = pool("p_xm")
    xm = [p_xm.tile([128, T], F32, name=f"xm{i}") for i in range(4)]

    def rms1_consumer(c, t, xn):
        dst = xm[c] if c < 4 else x1T[:, c]
        for b2 in range(2):
            b = 2 * t + b2
            nc.vector.tensor_scalar(dst[:, b * S:(b + 1) * S],
                                    xn[:, b2 * S:(b2 + 1) * S],
                                    scale1[:, c, b:b + 1],
                                    modT[:, 0 + c, b:b + 1],
                                    OP.mult, OP.add)

    r1_issue, _, r1_half, r1_release = rms_begin("r1", hsT, side="right")

    # ---------------- mod machinery (quad-batched ada DMAs) ------------------
    def mod_quad(o):
        """Computes modT chunks o..o+3 from one 1MB contiguous ada DMA."""
        wt = p_wq4.tile([128, HC, 512], BF16, tag="wq4", bufs=3, name="ada_t")
        nc.sync.dma_start(wt[:], d["ada_w"][o // 4])
        for pair in range(2):
            mp = ps_sc.tile([128, 2, BPC], F32, tag="sc", name="mod_ps")
            for k in range(2):
                cc = 2 * pair + k
                for f in range(HC):
                    nc.tensor.matmul(mp[:, k], wt[:, f, cc * 128:(cc + 1) * 128],
                                     stemb[:, f], start=(f == 0),
                                     stop=(f == HC - 1))
            nc.scalar.copy(modT[:, o + 2 * pair:o + 2 * pair + 2, :], mp[:])

    rp_holder = {}

    def rope_pair(pi):
        rp = rp_holder["rp"]
        (i0, i1, ct, st) = [(0, 1, "cxt", "sxt"), (2, 3, "cyt", "syt")][pi]
        a, bb = xm[i0], xm[i1]
        t1 = rp.tile([128, T], F32, tag="t1", bufs=2, name="t1")
        t2 = rp.tile([128, T], F32, tag="t2", bufs=2, name="t2")
        nc.vector.tensor_tensor(t1[:], a[:], rope_t[ct][:], OP.mult)
        nc.gpsimd.tensor_tensor(t2[:], bb[:], rope_t[st][:], OP.mult)
        nc.vector.tensor_tensor(x1T[:, i0], t1[:], t2[:], OP.subtract)
        t3 = rp.tile([128, T], F32, tag="t1", bufs=2, name="t3")
        t4 = rp.tile([128, T], F32, tag="t2", bufs=2, name="t4")
        nc.gpsimd.tensor_tensor(t3[:], a[:], rope_t[st][:], OP.mult)
        nc.vector.tensor_tensor(t4[:], bb[:], rope_t[ct][:], OP.mult)
        nc.vector.tensor_tensor(x1T[:, i1], t3[:], t4[:], OP.add)

    # ---------------- eva machinery (fp8 DoubleRow) --------------------------
    eva_state = {"wq": []}

    def eva_prefetch_w(o):
        wt = wev.tile([128, EC, 128], F8, tag="weva", name="eva_w_t")
        nc.sync.dma_start(wt[:], d["eva_w"][o])
        eva_state["wq"].append(wt)

    def eva_thunk(th, o, tq2):
        i = th * HC + o

        def run():
            with nc.named_scope("eva"):
                if tq2 == 0:
                    if i + 1 < 2 * HC:
                        eva_prefetch_w((i + 1) % HC)
                    eva_state["w"] = eva_state["wq"].pop(0)
                wt = eva_state["w"]
                tq = th * 2 + tq2
                p = ps_proj.tile([128, 256], F32, tag="proj", name="eva_ps")
                for f2 in range(EC // 2):
                    nc.tensor.matmul(p[:], wt[:, 2 * f2:2 * f2 + 2, :],
                                     ench[tq][:, 2 * f2:2 * f2 + 2, :],
                                     start=(f2 == 0), stop=(f2 == EC // 2 - 1),
                                     perf_mode=DR)
                nc.vector.tensor_scalar(evaT[:, o, tq * 256:(tq + 1) * 256],
                                        p[:], 1.0 / WSE,
                                        n_sb["eva_bT"][:, o:o + 1],
                                        OP.mult, OP.add)
        return run

    eva_prefetch_w(0)
    eva_q = [eva_thunk(th, o, tq2)
             for th in range(2) for o in range(HC) for tq2 in range(2)]

    # ---------------- phase 0: mod quads 0..15 + rms1 + rope + eva th0 -------
    with nc.named_scope("mod"):
        stemb = const.tile([128, HC, BPC], BF16)
        nc.scalar.activation(stemb[:], tembT_sb[:], AF.Silu)
        for q in range(4):
            mod_quad(4 * q)
            r1_issue(2 * q, on_vector=True)
            r1_issue(2 * q + 1, on_vector=True)
        nc.vector.tensor_tensor(modT[:, 0:16], modT[:, 0:16],
                                ada_bx4[:, 0:16], OP.add)
        for c in range(HC):
            nc.vector.tensor_scalar(scale1[:, c], modT[:, 8 + c], 1.0,
                                    n_sb["n1T"][:, c:c + 1],
                                    OP.add, OP.mult)
    with nc.named_scope("rms1"):
        r1_half(0, rms1_consumer)
        r1_half(1, rms1_consumer)
        r1_release()
    with nc.named_scope("rope"):
        rp_holder["rp"] = pool("p_ropetmp")
        rope_pair(0)
        rope_pair(1)
        free("p_ropetmp")
    free("p_xm")
    # eva token-half 0 on the PE while the vector tail above drains
    for _ in range(16):
        eva_q.pop(0)()
    free("p_rope")
    free("p_encA")

    # ---------------- helpers ----------------
    def proj_chunk(name, wt, src_sb, o, consumer, KC=HC, ts=(0, 1)):
        """One output chunk o of a Y^T projection (bf16 path)."""
        for t in ts:
            p = ps_proj.tile([128, 512], F32, tag="proj", name=f"{name}_ps")
            for f in range(KC):
                nc.tensor.matmul(p[:], wt[:, f],
                                 src_sb[:, f, t * 512:(t + 1) * 512],
                                 start=(f == 0), stop=(f == KC - 1))
            consumer(o, t, p)

    def proj_chunk8(name, wt, src_sb, o, consumer, KC=HC, ts=(0, 1)):
        """fp8 DoubleRow variant: weight tile [128, KC, 128] fp8, src fp8."""
        for t in ts:
            p = ps_proj.tile([128, 512], F32, tag="proj", name=f"{name}_ps")
            for f2 in range(KC // 2):
                nc.tensor.matmul(p[:], wt[:, 2 * f2:2 * f2 + 2, :],
                                 src_sb[:, 2 * f2:2 * f2 + 2,
                                        t * 512:(t + 1) * 512],
                                 start=(f2 == 0), stop=(f2 == KC // 2 - 1),
                                 perf_mode=DR)
            consumer(o, t, p)

    def proj_T(name, w_name, src_sb, KC, consumer, OC=HC, wtag="w8"):
        with nc.named_scope(name):
            for o in range(OC):
                wt = wpool.tile([128, KC, 128], BF16, tag=wtag, name=f"{name}_w")
                nc.sync.dma_start(wt[:], d[w_name][o])
                proj_chunk(name, wt, src_sb, o, consumer, KC=KC)

    def copy_act(dst):
        def c(o, t, p):
            nc.scalar.copy(dst[:, o, t * 512:(t + 1) * 512], p[:])
        return c

    def copy_act_s(dst, s):
        def c(o, t, p):
            nc.scalar.activation(dst[:, o, t * 512:(t + 1) * 512], p[:],
                                 AF.Copy, scale=s)
        return c

    def vnat_chunk(scope, wt, src_sb, oh, t, dst_v):
        """V natural chunk (bf16): token chunk t (128 toks), half oh."""
        p = ps_proj.tile([128, 512], F32, tag="proj", name=f"{scope}_ps")
        KC = src_sb.shape[1]
        for f in range(KC):
            nc.tensor.matmul(p[:], src_sb[:, f, t * 128:(t + 1) * 128],
                             wt[:, f], start=(f == 0), stop=(f == KC - 1))
        nc.scalar.copy(dst_v[:, t, oh * 512:(oh + 1) * 512], p[:])

    def vnat_chunk8(scope, wt, src_sb, oh, t, dst_v, s):
        """fp8 DoubleRow V natural chunk; drain rescales by s."""
        p = ps_proj.tile([128, 512], F32, tag="proj", name=f"{scope}_ps")
        KC = src_sb.shape[1]
        for f2 in range(KC // 2):
            nc.tensor.matmul(p[:],
                             src_sb[:, 2 * f2:2 * f2 + 2, t * 128:(t + 1) * 128],
                             wt[:, 2 * f2:2 * f2 + 2, :],
                             start=(f2 == 0), stop=(f2 == KC // 2 - 1),
                             perf_mode=DR)
        nc.scalar.activation(dst_v[:, t, oh * 512:(oh + 1) * 512], p[:],
                             AF.Copy, scale=s)

    def vnat(w_name, src_sb, dst_v, scope, side="left"):
        wv = pool("wv_" + scope, bufs=2, side=side)
        with nc.named_scope(scope):
            for oh in range(2):
                wt = wv.tile([128, HC, 512], BF16, tag="wvnat", name=f"{scope}_w")
                nc.sync.dma_start(wt[:], d[w_name][oh])
                for t in range(2 * BPC):
                    vnat_chunk(scope, wt, src_sb, oh, t, dst_v)
        free("wv_" + scope)

    # ---------------- attention unit machinery ----------------
    at_store = {}

    def attn_A(attnp, qt_sb, kt_sb, b, hc):
        at = []
        for ho in range(2):
            sc_ps = ps_sc.tile([128, 2, S], F32, tag="sc", name="sc_ps")
            for kc in range(2):
                nc.tensor.matmul(
                    sc_ps[:, kc],
                    kt_sb[ho * 64:(ho + 1) * 64, hc,
                          b * S + kc * 128: b * S + (kc + 1) * 128],
                    qt_sb[ho * 64:(ho + 1) * 64, hc, b * S:(b + 1) * S],
                    start=True, stop=True)
            a = attnp.tile([128, 2, S], BF16, tag=f"attn{ho}", bufs=2,
                           name="attn_sb")
            nc.scalar.activation(a[:], sc_ps[:], AF.Exp,
                                 scale=float(HD) ** -0.5)
            at.append(a)
        at_store[(b, hc)] = at

    def attn_B(attnp, vp_sb, out_sb, b, hc):
        at = at_store.pop((b, hc))
        for ho in range(2):
            h = 2 * hc + ho
            av = ps_av.tile([64, 512], F32, tag="av", name="av_ps")
            for kc in range(2):
                nc.tensor.matmul(av[:, 0:256],
                                 vp_sb[:, b * 2 + kc, h * 64:(h + 1) * 64],
                                 at[ho][:, kc],
                                 start=(kc == 0), stop=(kc == 1))
            for kc in range(2):
                nc.tensor.matmul(av[:, 256:512], ones_sb[:, 0:64],
                                 at[ho][:, kc],
                                 start=(kc == 0), stop=(kc == 1))
            inv = attnp.tile([64, S], F32, tag="inv", name="inv_sb")
            nc.vector.reciprocal_approx_fast(inv[:], av[:, 256:512])
            nc.vector.tensor_tensor(
                out_sb[ho * 64:(ho + 1) * 64, hc, b * S:(b + 1) * S],
                av[:, 0:256], inv[:], OP.mult)

    # ---------------- phase A: V1, Q1, K1 (dense bf16 GEMMs) -----------------
    p_vp = pool("p_vp")
    vp = p_vp.tile([128, 2 * BPC, NH * 64], BF16)
    p_encB = pool("p_encB", bufs=2)
    vnat("wv1", x1T, vp, "v1")
    # enc quarters 2,3 (needed by eva half 1 inside the attn1 window)
    for tq in range(2, 4):
        e_t = p_encB.tile([128, EC, 256], F8, tag="ench", name="ench")
        nc.scalar.dma_start(e_t[:], d["encT"][tq])
        ench.append(e_t)

    p_qt = pool("p_qt"); qt = p_qt.tile([128, HC, T], BF16)
    p_kt = pool("p_kt"); kt = p_kt.tile([128, HC, T], BF16)
    proj_T("q1", "wq1", x1T, HC, copy_act(qt))
    proj_T("k1", "wk1", x1T, HC, copy_act(kt))
    free("p_x1")

    # ---------------- phase B: attn1 || eva th1 || mod quads 16..47 ----------
    p_ao = pool("p_ao", side="right")
    attnout = p_ao.tile([128, HC, T], BF16)
    attnp1 = pool("attnp_attn1", bufs=3, side="right")

    def mod_thunk(o):
        def run():
            with nc.named_scope("mod"):
                mod_quad(o)
        return run

    def mod_final():
        with nc.named_scope("mod"):
            nc.vector.tensor_tensor(modT[:, 16:48], modT[:, 16:48],
                                    ada_bx4[:, 16:48], OP.add)
            for c in range(HC):
                nc.vector.tensor_scalar(scale3[:, c], modT[:, 32 + c], 1.0,
                                        n_sb["n3T"][:, c:c + 1],
                                        OP.add, OP.mult)

    # filler queue: 2 eva chunks then 1 mod quad, repeating (24 items for the
    # 24 filler slots among 32 units -- every 4th unit runs bare)
    fillers = []
    mq = [mod_thunk(o) for o in range(16, 48, 4)]
    for g in range(8):
        fillers.append(eva_q.pop(0))
        fillers.append(eva_q.pop(0))
        fillers.append(mq.pop(0))

    with nc.named_scope("attn1"):
        # batch-pair interleave: the partner unit's score MMs + fillers hide
        # each unit's exp latency
        for bp in (0, 2):
            for hc in range(NH // 2):
                attn_A(attnp1, qt, kt, bp, hc)
                attn_A(attnp1, qt, kt, bp + 1, hc)
                if fillers:
                    fillers.pop(0)()
                if hc % 2 == 0 and fillers:
                    fillers.pop(0)()
                attn_B(attnp1, vp, attnout, bp, hc)
                attn_B(attnp1, vp, attnout, bp + 1, hc)
        while fillers:
            fillers.pop(0)()
        mod_final()
    free("attnp_attn1")
    free("p_kt"); free("p_qt"); free("p_encB"); free("p_vp")
    free("p_wq4")
    free("p_weva")

    # ---------------- phase C: o1 (+rms2), rms2 tail -------------------------
    p_r2 = pool("p_r2")
    rms2T = p_r2.tile([128, HC, T], F8)
    r2_issue, _, r2_half, r2_release = rms_begin("r2", hsT, side="right")

    # prefetch cross-attention weights while o1 runs
    wv2 = pool("wv_v2", bufs=2)
    wv2_t = []
    for oh in range(2):
        wt = wv2.tile([128, HC, 512], F8, tag="wvnat", name="v2_w")
        nc.sync.dma_start(wt[:], d["wv2"][oh])
        wv2_t.append(wt)
    kq_tiles = []

    def kq_prefetch(hc):
        wtk = wpool.tile([128, HC, 128], F8, tag="w8f", name="k2_w")
        nc.sync.dma_start(wtk[:], d["wk2"][hc])
        wtq = wpool.tile([128, HC, 128], F8, tag="w8f", name="q2_w")
        nc.sync.dma_start(wtq[:], d["wq2"][hc])
        kq_tiles.append((wtk, wtq))

    kq_prefetch(0)

    def resid_gated(g_split, rms_issue=None):
        def c(o, t, p):
            tg = sml.tile([128, 512], F32, tag="resid", name="resid_t")
            for b2 in range(2):
                b = t * 2 + b2
                nc.vector.tensor_scalar(tg[:, b2 * S:(b2 + 1) * S],
                                        p[:, b2 * S:(b2 + 1) * S],
                                        modT[:, g_split * 8 + o, b:b + 1],
                                        None, OP.mult)
            nc.vector.tensor_tensor(hsT[:, o, t * 512:(t + 1) * 512],
                                    hsT[:, o, t * 512:(t + 1) * 512],
                                    tg[:], OP.add)
            if t == 1 and rms_issue is not None:
                rms_issue(o)
        return c

    proj_T("o1", "wo1", attnout, HC, resid_gated(2, r2_issue))

    def rms2_consumer(c, t, xn):
        nc.vector.tensor_scalar(rms2T[:, c, t * 512:(t + 1) * 512], xn[:],
                                n_sb["n2T"][:, c:c + 1], None, OP.mult)

    with nc.named_scope("rms2"):
        r2_half(0, rms2_consumer)   # hsT now holds h1
        r2_half(1, rms2_consumer)
    r2_release()
    free("p_ao")

    # ---------------- phase D: cross attention || k2/q2/v2 (fp8) -------------
    p_ao2 = pool("p_ao2", side="right")
    attn2out = p_ao2.tile([128, HC, T], F8)
    attnp2 = pool("attnp_attn2", bufs=3, side="right")
    p_k2 = pool("p_k2", side="right"); k2t = p_k2.tile([128, HC, T], BF16)
    p_q2 = pool("p_q2", side="right"); q2t = p_q2.tile([128, HC, T], BF16)
    p_v2 = pool("p_v2", side="right")
    vp2 = p_v2.tile([128, 2 * BPC, NH * 64], BF16)

    with nc.named_scope("attn2"):
        # V for b0+b1 (token chunks 0..3, both halves) up front
        for t in range(4):
            for oh in range(2):
                vnat_chunk8("v2", wv2_t[oh], evaT, oh, t, vp2, 1.0 / WS2)
        # b0/b1 unit pairs pipelined with k2/q2 chunk emission; V chunks for
        # b2/b3 ride along as extra filler
        vfill = [(t, oh) for t in range(4, 2 * BPC) for oh in range(2)]
        for hc in range(HC):
            if hc + 1 < HC:
                kq_prefetch(hc + 1)
            wtk, wtq = kq_tiles.pop(0)
            proj_chunk8("k2", wtk, evaT, hc, copy_act_s(k2t, 1.0 / WS2))
            proj_chunk8("q2", wtq, rms2T, hc, copy_act_s(q2t, 1.0 / WS2))
            if hc >= 1:
                attn_B(attnp2, vp2, attn2out, 0, hc - 1)
                attn_B(attnp2, vp2, attn2out, 1, hc - 1)
            attn_A(attnp2, q2t, k2t, 0, hc)
            attn_A(attnp2, q2t, k2t, 1, hc)
            if vfill:
                t, oh = vfill.pop(0)
                vnat_chunk8("v2", wv2_t[oh], evaT, oh, t, vp2, 1.0 / WS2)
        attn_B(attnp2, vp2, attn2out, 0, HC - 1)
        attn_B(attnp2, vp2, attn2out, 1, HC - 1)
    free("wv_v2")
    free("p_r2")
    free("p_eva")

    # ---------------- phase E: o2/rms3 by token halves, then MLP -------------
    p_y = pool("p_y")
    yT = p_y.tile([128, HC, T], BF16)
    _, r3_issue_h, r3_half, _ = rms_begin("r3", hsT, side="left")

    wo2p = pool("p_wo2", bufs=1)
    wo2_t = wo2p.tile([128, HC, HC, 128], F8)
    for o in range(HC):
        nc.sync.dma_start(wo2_t[:, o], d["wo2"][o])

    def resid_plain(o, t, p):
        tmp = sml.tile([128, 512], F32, tag="o2s", bufs=2, name="o2s")
        nc.scalar.activation(tmp[:], p[:], AF.Copy, scale=1.0 / WS2)
        nc.vector.tensor_tensor(hsT[:, o, t * 512:(t + 1) * 512],
                                hsT[:, o, t * 512:(t + 1) * 512],
                                tmp[:], OP.add)
        r3_issue_h(o, t)

    def rms3_consumer(c, t, xn):
        for b2 in range(2):
            b = 2 * t + b2
            nc.vector.tensor_scalar(yT[:, c, b * S:(b + 1) * S],
                                    xn[:, b2 * S:(b2 + 1) * S],
                                    scale3[:, c, b:b + 1],
                                    modT[:, 24 + c, b:b + 1],
                                    OP.mult, OP.add)

    with nc.named_scope("attn2"):
        # b2/b3 unit pairs with o2(t0) chunks as filler
        for hc in range(HC):
            attn_A(attnp2, q2t, k2t, 2, hc)
            attn_A(attnp2, q2t, k2t, 3, hc)
            with nc.named_scope("o2"):
                proj_chunk8("o2", wo2_t[:, hc], attn2out, hc, resid_plain,
                            ts=(0,))
            attn_B(attnp2, vp2, attn2out, 2, hc)
            attn_B(attnp2, vp2, attn2out, 3, hc)
    # q2/k2/v2 and the exp tiles are dead once the b3 units are emitted;
    # free (LIFO) before the MLP reserves its 64KB/partition of mlpT slots.
    free("p_v2"); free("p_q2"); free("p_k2"); free("attnp_attn2")

    # rms3 half0 vector tail hides under the o2(t1) PE stream
    with nc.named_scope("rms3"):
        r3_half(0, rms3_consumer)
    with nc.named_scope("o2"):
        for o in range(HC):
            proj_chunk8("o2", wo2_t[:, o], attn2out, o, resid_plain, ts=(1,))
    free("p_ao2")
    with nc.named_scope("rms3"):
        r3_half(1, rms3_consumer)

    # ---------------- MLP: gate/up/down, weights shared across halves --------
    out_r = r3(outT_d)
    p_mlp = pool("p_mlp")
    wmlp = pool("p_wmlp", bufs=5)
    wdn = pool("p_wdown", bufs=2)
    mlpT = [p_mlp.tile([128, IC, 512], BF16, tag="mlp", bufs=2,
                       name=f"mlpT{t}") for t in range(2)]

    def gu_mm(wg, wu, oc, ts):
        for t in ts:
            pg = ps_proj.tile([128, 512], F32, tag="proj", name="g_ps")
            for f in range(HC):
                nc.tensor.matmul(pg[:], wg[:, f],
                                 yT[:, f, t * 512:(t + 1) * 512],
                                 start=(f == 0), stop=(f == HC - 1))
            pu = ps_proj.tile([128, 512], F32, tag="proj", name="u_ps")
            for f in range(HC):
                nc.tensor.matmul(pu[:], wu[:, f],
                                 yT[:, f, t * 512:(t + 1) * 512],
                                 start=(f == 0), stop=(f == HC - 1))
            gs = sml.tile([128, 512], BF16, tag="gsil", name="gsil")
            nc.scalar.activation(gs[:], pg[:], AF.Silu)
            nc.vector.tensor_tensor(mlpT[t][:, oc], gs[:], pu[:], OP.mult)

    def gu_chunk(oc, ts):
        wg = wmlp.tile([128, HC, 128], BF16, tag="w8", name="gate_w_t")
        nc.sync.dma_start(wg[:], d["gate_w"][oc])
        wu = wmlp.tile([128, HC, 128], BF16, tag="w8", name="up_w_t")
        nc.sync.dma_start(wu[:], d["up_w"][oc])
        gu_mm(wg, wu, oc, ts)

    with nc.named_scope("gateup0"):
        for oc in range(4):
            gu_chunk(oc, (0,))
    with nc.named_scope("gateup1"):
        for oc in range(4, IC):
            gu_chunk(oc, (0, 1))
        for oc in range(4):
            gu_chunk(oc, (1,))

    with nc.named_scope("down0"):
        for o in range(HC):
            wt = wdn.tile([128, IC, 128], BF16, tag="wdown", name="down_w_t")
            nc.sync.dma_start(wt[:], d["down_w"][o])
            for t in range(2):
                p = ps_proj.tile([128, 512], F32, tag="proj", name="d_ps")
                for f in range(IC):
                    nc.tensor.matmul(p[:], wt[:, f], mlpT[t][:, f],
                                     start=(f == 0), stop=(f == IC - 1))
                sl = slice(t * 512, (t + 1) * 512)
                ob = sml.tile([128, 512], F32, tag="outb", bufs=2, name="outb")
                for b2 in range(2):
                    b = 2 * t + b2
                    nc.vector.tensor_scalar(ob[:, b2 * S:(b2 + 1) * S],
                                            p[:, b2 * S:(b2 + 1) * S],
                                            modT[:, 40 + o, b:b + 1],
                                            None, OP.mult)
                nc.vector.tensor_tensor(ob[:], ob[:], hsT[:, o, sl], OP.add)
                nc.sync.dma_start(out_r[:, o, sl], ob[:])

    for nm in reversed(list(open_pools)):
        free(nm)
    for p in list(ms_pools.values()):
        p.release()
    ps_av.release(); ps_sc.release(); ps_proj.release()
    tc_cm.__exit__(None, None, None)
    nc.compile()
    return nc


_CACHE = {}


def _get_program():
    if "nc" not in _CACHE:
        _CACHE["nc"] = build_program()
    return _CACHE["nc"]


def kernel(hidden_states, encoder_hidden_states, timestep_emb,
           wq1, wk1, wv1, wo1, wq2, wk2, wv2, wo2,
           eva_w, eva_b, ada_w, ada_b, gate_w, up_w, down_w, n1, n2, n3,
           _trace=False):
    nc = _get_program()
    f32 = lambda a: np.ascontiguousarray(np.asarray(a), dtype=np.float32)

    def packw(w, cols, dtnp, scale=None):
        """[K, N] weight -> tile-major [N//cols, 128, K//128, cols] so each
        output-chunk weight DMA is one contiguous DRAM read."""
        w = np.asarray(w, np.float32)
        if scale is not None:
            w = w * scale
        K, N = w.shape
        p = w.reshape(K // 128, 128, N // cols, cols).transpose(2, 1, 0, 3)
        return np.ascontiguousarray(p).astype(dtnp)

    cxt, sxt, cyt, syt = _rope_tables()
    ropeT = np.ascontiguousarray(np.stack([cxt, sxt, cyt, syt], axis=1))
    colchunks = lambda v, n: np.asarray(v, np.float32).reshape(n, 128).T
    ada_bT = colchunks(ada_b, 48)
    constsF = np.concatenate([
        colchunks(n1, HC), colchunks(n2, HC), colchunks(n3, HC),
        colchunks(eva_b, HC), ada_bT,
        np.full((128, 1), EPS, np.float32),
        np.repeat(ada_bT, 4, axis=1)], axis=1)
    shared = dict(
        wq1=packw(wq1, 128, BF16NP), wk1=packw(wk1, 128, BF16NP),
        wv1=packw(wv1, 512, BF16NP), wo1=packw(wo1, 128, BF16NP),
        wq2=packw(wq2, 128, F8NP, WS2), wk2=packw(wk2, 128, F8NP, WS2),
        wv2=packw(wv2, 512, F8NP, WS2), wo2=packw(wo2, 128, F8NP, WS2),
        eva_w=packw(eva_w, 128, F8NP, WSE),
        ada_w=packw(ada_w, 512, BF16NP),
        gate_w=packw(gate_w, 128, BF16NP), up_w=packw(up_w, 128, BF16NP),
        down_w=packw(down_w, 128, BF16NP),
        constsF=np.ascontiguousarray(constsF),
        ropeT=ropeT,
        ones=np.ones((128, 128), BF16NP),
    )
    hs = f32(hidden_states)
    enc = f32(encoder_hidden_states)
    temb = f32(timestep_emb)

    in_maps = []
    for c in range(NC_):
        sl = slice(c * BPC, (c + 1) * BPC)
        m = dict(shared)
        m["hsT"] = np.ascontiguousarray(hs[sl].transpose(2, 0, 1).reshape(H, T))
        m["encT"] = packw(enc[sl].transpose(2, 0, 1).reshape(E, T), 256, F8NP)
        m["tembT"] = np.ascontiguousarray(temb[sl].T)
        in_maps.append(m)

    res = run_bass_kernel_spmd(nc, in_maps, core_ids=list(range(NC_)),
                               trace=_trace)
    out = np.empty((B, S, H), np.float32)
    for c in range(NC_):
        o = res.results[c]["outT"]  # [H, T]
        out[c * BPC:(c + 1) * BPC] = np.ascontiguousarray(o.T).reshape(BPC, S, H)
    if _trace:
        kernel.last_results = res
    return out


# revision 59
# speedup vs baseline: 1.0625x; 1.0296x over previous
"""BLIP3o DiT block on 8 Trainium2 NeuronCores.

Strategy: data-parallel over batch (32 batches -> 4 per core), zero collectives.
On-chip layout: activations live transposed [feature, token]; matmul operands
are bf16, except the cross-attention side (eva / q2 / k2 / v2 / o2) which runs
fp8e4 with DoubleRow (2 contraction rows per PE cell, ~1.8x) -- weights are
pre-scaled by 32/64 on the host to clear the fp8 subnormal floor and the
inverse scale is folded into each PSUM drain.  Accumulation stays fp32 in
PSUM; the residual stream (hsT), modulation, rms statistics and rope tables
stay fp32.

PE-density structure (HAM clock-gate driven): every softmax-latency bubble is
filled with independent GEMM work so the PE never idles long enough to
re-throttle to K=4/8:
  phase 0   mod chunks 0..15 (quad-batched ada DMAs); rms1 + rope on vector;
            eva token-half 0 (fp8) covers the vector tail on the PE
  phase A   v1, q1, k1 (dense bf16)
  phase B   attn1 units interleaved with eva half 1 + deferred mod quads
  phase C   o1 (+rms2 issue), rms2 tail
  phase D   attn2 units interleaved with k2/q2/v2 fp8 chunk emission
  phase E   o2(t0) rides the b2 units; o2(t1) + rms3 precede the MLP;
            gate/up/down share one weight DMA across both token halves
            (catch-up re-loads only the first 4 gate/up chunks); down
            accumulates full-I in one PSUM bank; final gate/residual/store
            folded into the down drain.
Softmax denominators accumulate into the spare half of the AV PSUM bank.
Input DMAs ride the Activation HWDGE ring; weight streams ride the SP ring.
"""
import os
import sys
import numpy as np

if "/root/pylocal" not in sys.path:
    sys.path.insert(0, "/root/pylocal")  # antenv.axon_hooks shim (NTFF tracing)
try:
    import antenv
    if "/root/pylocal/antenv" not in list(antenv.__path__):
        antenv.__path__.append("/root/pylocal/antenv")
except Exception:
    pass

import concourse.bass as bass
from concourse import bacc
import concourse.mybir as mybir
from concourse.tile import TileContext
from concourse.bass_utils import run_bass_kernel_spmd

F32 = mybir.dt.float32
BF16 = mybir.dt.bfloat16
F8 = mybir.dt.float8e4
AF = mybir.ActivationFunctionType
OP = mybir.AluOpType
DR = mybir.MatmulPerfMode.DoubleRow
BF16NP = mybir.dt.np(mybir.dt.bfloat16)
F8NP = mybir.dt.np(F8)

B, S, L, H, NH, HD, I, E = 32, 256, 256, 1024, 16, 64, 4096, 4096
EPS = 1e-6
GRID = 16
NC_ = 8            # cores
BPC = B // NC_     # batches per core = 4
T = BPC * S        # tokens per core = 1024
HC = H // 128      # 8 feature chunks
EC = E // 128      # 32
IC = I // 128      # 32
NCONST = 3 * HC + HC + 48 + 1 + 192   # n1T n2T n3T eva_bT ada_bT eps ada_bT_x4
WS2 = 32.0         # host pre-scale on wq2/wk2/wv2/wo2 (fp8 range placement)
WSE = 64.0         # host pre-scale on eva_w


def _rope_tables():
    q = H // 4
    inv = 1.0 / (10000.0 ** (np.arange(0, q, dtype=np.float64)[::2] / q))  # [128]
    qd = 128
    pos_x = np.repeat(np.arange(GRID, dtype=np.float64), GRID)  # [S]
    pos_y = np.tile(np.arange(GRID, dtype=np.float64), GRID)
    fx = pos_x[:, None] * inv[None, :qd]   # [S, 128]
    fy = pos_y[:, None] * inv[None, :qd]
    t = lambda a: np.ascontiguousarray(
        np.tile(a.T.astype(np.float32), (1, BPC)))  # [128, S] -> [128, T]
    return t(np.cos(fx)), t(np.sin(fx)), t(np.cos(fy)), t(np.sin(fy))


def build_program():
    nc = bacc.Bacc()

    # ---------------- DRAM params ----------------
    d = {}
    def P(name, shape, dt, out=False):
        d[name] = nc.declare_dram_parameter(name, list(shape), dt, isOutput=out)
        return d[name]

    # weights are host-packed tile-major [OC, 128, KC, cols] so every weight
    # DMA is one fully-contiguous read (the (c p) rearrange view otherwise
    # yields 128-256B DRAM runs and ~50-150GB/s effective DMA)
    hsT_d = P("hsT", [H, T], F32)
    P("encT", [4, 128, EC, 256], F8)
    P("tembT", [H, BPC], F32)
    for w in ["wq1", "wk1", "wo1"]:
        P(w, [HC, 128, HC, 128], BF16)
    P("wv1", [2, 128, HC, 512], BF16)
    for w in ["wq2", "wk2", "wo2"]:
        P(w, [HC, 128, HC, 128], F8)
    P("wv2", [2, 128, HC, 512], F8)
    P("eva_w", [HC, 128, EC, 128], F8)
    P("ada_w", [12, 128, HC, 512], BF16)
    P("gate_w", [IC, 128, HC, 128], BF16)
    P("up_w", [IC, 128, HC, 128], BF16)
    P("down_w", [HC, 128, IC, 128], BF16)
    P("constsF", [128, NCONST], F32)   # n1T | n2T | n3T | eva_bT | ada_bT | eps
    P("ropeT", [128, 4, T], F32)       # cxt | sxt | cyt | syt
    P("ones", [128, 128], BF16)
    outT_d = P("outT", [H, T], F32, out=True)

    r3 = lambda ap: ap.rearrange("(c p) t -> p c t", p=128)

    tc_cm = TileContext(nc)
    tc = tc_cm.__enter__()

    open_pools = {}

    def pool(name, bufs=1, side="left"):
        p = tc.alloc_tile_pool(name=name, bufs=bufs, side=side)
        open_pools[name] = p
        return p

    def free(name):
        open_pools.pop(name).release()

    # long-lived small pools
    wpool = pool("wstream", bufs=4)         # tags: w8 (bf16), w8f (fp8), wq4
    sml = pool("sml", bufs=1)               # resid/gsil/outb/o2s tags
    const = pool("const", bufs=1)
    ps_proj = tc.alloc_tile_pool(name="ps_proj", bufs=2, space="PSUM")
    ps_sc = tc.alloc_tile_pool(name="ps_sc", bufs=2, space="PSUM")
    ps_av = tc.alloc_tile_pool(name="ps_av", bufs=2, space="PSUM")

    # ---------------- constants + input DMA kickoff ----------------
    tembT_sb = const.tile([128, HC, BPC], F32)
    nc.sync.dma_start(tembT_sb[:], d["tembT"].rearrange("(c p) b -> p c b", p=128))
    cst = const.tile([128, NCONST], F32)
    nc.sync.dma_start(cst[:], d["constsF"][:])
    n_sb = {"n1T": cst[:, 0:8], "n2T": cst[:, 8:16], "n3T": cst[:, 16:24],
            "eva_bT": cst[:, 24:32], "ada_bT": cst[:, 32:80]}
    eps_sb = cst[:, 80:81]
    ada_bx4 = cst[:, 81:81 + 192].rearrange("p (o b) -> p o b", b=BPC)
    ones_sb = const.tile([128, 128], BF16)
    nc.sync.dma_start(ones_sb[:], d["ones"][:])

    modT = const.tile([128, 48, BPC], F32)      # 6 splits x 8 chunks
    scale1 = const.tile([128, HC, BPC], F32)    # n1*(1+sc_msa)
    scale3 = const.tile([128, HC, BPC], F32)    # n3*(1+sc_mlp)

    # hsT / rope / enc quarters ride the Activation HWDGE ring (parallel to
    # the SP ring carrying ada/weight streams)
    p_hs = pool("p_hs")
    hsT = p_hs.tile([128, HC, T], F32)          # becomes h1T, then h2T in place
    hs_r = r3(hsT_d)
    nc.scalar.dma_start(hsT[:, 0:4], hs_r[:, 0:4])
    nc.scalar.dma_start(hsT[:, 4:8], hs_r[:, 4:8])

    p_eva = pool("p_eva")
    evaT = p_eva.tile([128, HC, T], F8)
    wev = pool("p_weva", bufs=4)
    p_wq4 = pool("p_wq4", bufs=3)   # mod-quad ada tiles; freed after attn1
    p_encA = pool("p_encA", bufs=2)  # quarters 0,1 (eva half 0); freed early
    ench = []
    for tq in range(2):   # quarters 2,3 DMA'd after phase A kickoff
        e_t = p_encA.tile([128, EC, 256], F8, tag="ench", name="ench")
        nc.scalar.dma_start(e_t[:], d["encT"][tq])
        ench.append(e_t)

    p_rope = pool("p_rope")
    ropeT_sb = p_rope.tile([128, 4, T], F32)
    nc.scalar.dma_start(ropeT_sb[:], d["ropeT"][:])
    rope_t = {tb: ropeT_sb[:, i]
              for i, tb in enumerate(["cxt", "sxt", "cyt", "syt"])}

    ms_pools = {}

    # ---------------- rms machinery ----------------
    def rms_begin(name, x_sb, side="left"):
        rtmp = pool("rtmp_" + name, side=side)
        ps_ms = tc.alloc_tile_pool(name="ps_ms_" + name, bufs=1, space="PSUM")
        ms_pools[name] = ps_ms
        ms = [ps_ms.tile([128, 512], F32, name=f"ms_{name}_{t}")
              for t in range(2)]

        def issue(c, on_vector=False):
            sq = rtmp.tile([128, T], BF16, tag="sq", bufs=2, name=f"sq_{name}")
            if on_vector:
                nc.vector.tensor_tensor(sq[:], x_sb[:, c], x_sb[:, c], OP.mult)
            else:
                nc.scalar.activation(sq[:], x_sb[:, c], AF.Square)
            for t in range(2):
                nc.tensor.matmul(ms[t][:], ones_sb[:],
                                 sq[:, t * 512:(t + 1) * 512],
                                 start=(c == 0), stop=(c == HC - 1))

        def issue_half(c, t, on_vector=False):
            sq = rtmp.tile([128, 512], BF16, tag=f"sqh{t}", bufs=2,
                           name=f"sqh_{name}")
            src = x_sb[:, c, t * 512:(t + 1) * 512]
            if on_vector:
                nc.vector.tensor_tensor(sq[:], src, src, OP.mult)
            else:
                nc.scalar.activation(sq[:], src, AF.Square)
            nc.tensor.matmul(ms[t][:], ones_sb[:], sq[:],
                             start=(c == 0), stop=(c == HC - 1))

        def finish_half(t, consumer):
            sroot = rtmp.tile([128, 512], F32, tag="sroot", bufs=2,
                              name=f"sroot_{name}")
            nc.scalar.activation(sroot[:], ms[t][:], AF.Sqrt,
                                 bias=eps_sb, scale=1.0 / H)
            invn = rtmp.tile([128, 512], F32, tag="invn", bufs=2,
                             name=f"invn_{name}")
            nc.vector.reciprocal_approx_fast(invn[:], sroot[:])
            for c in range(HC):
                xn = rtmp.tile([128, 512], F32, tag="xn", bufs=3,
                               name=f"xn_{name}")
                nc.vector.tensor_tensor(xn[:],
                                        x_sb[:, c, t * 512:(t + 1) * 512],
                                        invn[:], OP.mult)
                consumer(c, t, xn)

        def release():
            ms_pools.pop(name).release()
            free("rtmp_" + name)

        return issue, issue_half, finish_half, release

    # x1T / rope staging (rms1 consumers write here during the mod loop)
    p_x1 = pool("p_x1", side="right")
    x1T = p_x1.tile([128, HC, T], BF16)
    p_xm = pool("p_xm")
    xm = [p_xm.tile([128, T], F32, name=f"xm{i}") for i in range(4)]

    def rms1_consumer(c, t, xn):
        dst = xm[c] if c < 4 else x1T[:, c]
        for b2 in range(2):
            b = 2 * t + b2
            nc.vector.tensor_scalar(dst[:, b * S:(b + 1) * S],
                                    xn[:, b2 * S:(b2 + 1) * S],
                                    scale1[:, c, b:b + 1],
                                    modT[:, 0 + c, b:b + 1],
                                    OP.mult, OP.add)

    r1_issue, _, r1_half, r1_release = rms_begin("r1", hsT, side="right")

    # ---------------- mod machinery (quad-batched ada DMAs) ------------------
    def mod_quad(o, eng=None):
        """Computes modT chunks o..o+3 from one 1MB contiguous ada DMA.
        Inside the attn1 window the DMA rides the Act ring: the act
        sequencer is paced by the exps, so the tile-slot wait is already
        satisfied at dispatch and doesn't convoy the SP ring."""
        wt = p_wq4.tile([128, HC, 512], BF16, tag="wq4", bufs=3, name="ada_t")
        (eng or nc.sync).dma_start(wt[:], d["ada_w"][o // 4])
        for pair in range(2):
            mp = ps_sc.tile([128, 2, BPC], F32, tag="sc", name="mod_ps")
            for k in range(2):
                cc = 2 * pair + k
                for f in range(HC):
                    nc.tensor.matmul(mp[:, k], wt[:, f, cc * 128:(cc + 1) * 128],
                                     stemb[:, f], start=(f == 0),
                                     stop=(f == HC - 1))
            nc.scalar.copy(modT[:, o + 2 * pair:o + 2 * pair + 2, :], mp[:])

    rp_holder = {}

    def rope_pair(pi):
        rp = rp_holder["rp"]
        (i0, i1, ct, st) = [(0, 1, "cxt", "sxt"), (2, 3, "cyt", "syt")][pi]
        a, bb = xm[i0], xm[i1]
        t1 = rp.tile([128, T], F32, tag="t1", bufs=2, name="t1")
        t2 = rp.tile([128, T], F32, tag="t2", bufs=2, name="t2")
        nc.vector.tensor_tensor(t1[:], a[:], rope_t[ct][:], OP.mult)
        nc.gpsimd.tensor_tensor(t2[:], bb[:], rope_t[st][:], OP.mult)
        nc.vector.tensor_tensor(x1T[:, i0], t1[:], t2[:], OP.subtract)
        t3 = rp.tile([128, T], F32, tag="t1", bufs=2, name="t3")
        t4 = rp.tile([128, T], F32, tag="t2", bufs=2, name="t4")
        nc.gpsimd.tensor_tensor(t3[:], a[:], rope_t[st][:], OP.mult)
        nc.vector.tensor_tensor(t4[:], bb[:], rope_t[ct][:], OP.mult)
        nc.vector.tensor_tensor(x1T[:, i1], t3[:], t4[:], OP.add)

    # ---------------- eva machinery (fp8 DoubleRow) --------------------------
    eva_state = {"wq": []}

    def eva_prefetch_w(o, eng=None):
        wt = wev.tile([128, EC, 128], F8, tag="weva", name="eva_w_t")
        (eng or nc.sync).dma_start(wt[:], d["eva_w"][o])
        eva_state["wq"].append(wt)

    def eva_thunk(th, o, tq2):
        i = th * HC + o

        def run():
            with nc.named_scope("eva"):
                if tq2 == 0:
                    eva_state["w"] = eva_state["wq"].pop(0)
                wt = eva_state["w"]
                tq = th * 2 + tq2
                p = ps_proj.tile([128, 256], F32, tag="proj", name="eva_ps")
                for f2 in range(EC // 2):
                    nc.tensor.matmul(p[:], wt[:, 2 * f2:2 * f2 + 2, :],
                                     ench[tq][:, 2 * f2:2 * f2 + 2, :],
                                     start=(f2 == 0), stop=(f2 == EC // 2 - 1),
                                     perf_mode=DR)
                # drain on ScalarE: keeps eva's PSUM rotation off the deep
                # rms/rope vector tail in phase 0
                nc.scalar.activation(evaT[:, o, tq * 256:(tq + 1) * 256],
                                     p[:], AF.Identity, scale=1.0 / WSE,
                                     bias=n_sb["eva_bT"][:, o:o + 1])
                if tq2 == 1 and i + 4 < 2 * HC:
                    # prefetch AFTER w(i)'s last reads: w(i+4) reuses its
                    # slot.  Half-1 thunks run inside the attn1 window ->
                    # their prefetches ride the (exp-paced) Act ring.
                    eva_prefetch_w((i + 4) % HC,
                                   nc.scalar if i + 4 >= 12 else None)
        return run

    for o4 in range(4):
        eva_prefetch_w(o4)
    eva_q = [eva_thunk(th, o, tq2)
             for th in range(2) for o in range(HC) for tq2 in range(2)]

    # ---------------- phase 0: mod quads 0..15 + rms1 + rope + eva th0 -------
    with nc.named_scope("mod"):
        stemb = const.tile([128, HC, BPC], BF16)
        nc.scalar.activation(stemb[:], tembT_sb[:], AF.Silu)
        mod_quad(0)
        mod_quad(4)
        for c in range(4):
            r1_issue(c, on_vector=True)
        mod_quad(8)
        mod_quad(12)
        for c in range(4, 8):
            r1_issue(c, on_vector=True)
        nc.vector.tensor_tensor(modT[:, 0:16], modT[:, 0:16],
                                ada_bx4[:, 0:16], OP.add)
        for c in range(HC):
            nc.vector.tensor_scalar(scale1[:, c], modT[:, 8 + c], 1.0,
                                    n_sb["n1T"][:, c:c + 1],
                                    OP.add, OP.mult)
    with nc.named_scope("rms1"):
        r1_half(0, rms1_consumer)
        r1_half(1, rms1_consumer)
        r1_release()
    with nc.named_scope("rope"):
        rp_holder["rp"] = pool("p_ropetmp")
        rope_pair(0)
        rope_pair(1)
        free("p_ropetmp")
    free("p_xm")
    # eva token-half 0 on the PE while the vector tail above drains
    for _ in range(16):
        eva_q.pop(0)()
    free("p_rope")
    free("p_encA")

    # ---------------- helpers ----------------
    def proj_chunk(name, wt, src_sb, o, consumer, KC=HC, ts=(0, 1)):
        """One output chunk o of a Y^T projection (bf16 path)."""
        for t in ts:
            p = ps_proj.tile([128, 512], F32, tag="proj", name=f"{name}_ps")
            for f in range(KC):
                nc.tensor.matmul(p[:], wt[:, f],
                                 src_sb[:, f, t * 512:(t + 1) * 512],
                                 start=(f == 0), stop=(f == KC - 1))
            consumer(o, t, p)

    def proj_chunk8(name, wt, src_sb, o, consumer, KC=HC, ts=(0, 1)):
        """fp8 DoubleRow variant: weight tile [128, KC, 128] fp8, src fp8."""
        for t in ts:
            p = ps_proj.tile([128, 512], F32, tag="proj", name=f"{name}_ps")
            for f2 in range(KC // 2):
                nc.tensor.matmul(p[:], wt[:, 2 * f2:2 * f2 + 2, :],
                                 src_sb[:, 2 * f2:2 * f2 + 2,
                                        t * 512:(t + 1) * 512],
                                 start=(f2 == 0), stop=(f2 == KC // 2 - 1),
                                 perf_mode=DR)
            consumer(o, t, p)

    def proj_T(name, w_name, src_sb, KC, consumer, OC=HC, wtag="w8"):
        with nc.named_scope(name):
            for o in range(OC):
                wt = wpool.tile([128, KC, 128], BF16, tag=wtag, name=f"{name}_w")
                nc.sync.dma_start(wt[:], d[w_name][o])
                proj_chunk(name, wt, src_sb, o, consumer, KC=KC)

    def copy_act(dst):
        def c(o, t, p):
            nc.scalar.copy(dst[:, o, t * 512:(t + 1) * 512], p[:])
        return c

    def copy_act_s(dst, s):
        def c(o, t, p):
            nc.scalar.activation(dst[:, o, t * 512:(t + 1) * 512], p[:],
                                 AF.Copy, scale=s)
        return c

    def vnat_chunk(scope, wt, src_sb, oh, t, dst_v):
        """V natural chunk (bf16): token chunk t (128 toks), half oh."""
        p = ps_proj.tile([128, 512], F32, tag="proj", name=f"{scope}_ps")
        KC = src_sb.shape[1]
        for f in range(KC):
            nc.tensor.matmul(p[:], src_sb[:, f, t * 128:(t + 1) * 128],
                             wt[:, f], start=(f == 0), stop=(f == KC - 1))
        nc.scalar.copy(dst_v[:, t, oh * 512:(oh + 1) * 512], p[:])

    def vnat_chunk8(scope, wt, src_sb, oh, t, dst_v, s):
        """fp8 DoubleRow V natural chunk; drain rescales by s."""
        p = ps_proj.tile([128, 512], F32, tag="proj", name=f"{scope}_ps")
        KC = src_sb.shape[1]
        for f2 in range(KC // 2):
            nc.tensor.matmul(p[:],
                             src_sb[:, 2 * f2:2 * f2 + 2, t * 128:(t + 1) * 128],
                             wt[:, 2 * f2:2 * f2 + 2, :],
                             start=(f2 == 0), stop=(f2 == KC // 2 - 1),
                             perf_mode=DR)
        nc.scalar.activation(dst_v[:, t, oh * 512:(oh + 1) * 512], p[:],
                             AF.Copy, scale=s)

    def vnat(w_name, src_sb, dst_v, scope, side="left"):
        wv = pool("wv_" + scope, bufs=2, side=side)
        with nc.named_scope(scope):
            for oh in range(2):
                wt = wv.tile([128, HC, 512], BF16, tag="wvnat", name=f"{scope}_w")
                nc.sync.dma_start(wt[:], d[w_name][oh])
                for t in range(2 * BPC):
                    vnat_chunk(scope, wt, src_sb, oh, t, dst_v)
        free("wv_" + scope)

    # ---------------- attention unit machinery ----------------
    at_store = {}

    def attn_A(attnp, qt_sb, kt_sb, b, hc):
        at = []
        for ho in range(2):
            sc_ps = ps_sc.tile([128, 2, S], F32, tag="sc", name="sc_ps")
            for kc in range(2):
                nc.tensor.matmul(
                    sc_ps[:, kc],
                    kt_sb[ho * 64:(ho + 1) * 64, hc,
                          b * S + kc * 128: b * S + (kc + 1) * 128],
                    qt_sb[ho * 64:(ho + 1) * 64, hc, b * S:(b + 1) * S],
                    start=True, stop=True)
            a = attnp.tile([128, 2, S], BF16, tag=f"attn{ho}", bufs=2,
                           name="attn_sb")
            nc.scalar.activation(a[:], sc_ps[:], AF.Exp,
                                 scale=float(HD) ** -0.5)
            at.append(a)
        at_store[(b, hc)] = at

    def attn_B(attnp, vp_sb, out_sb, b, hc):
        at = at_store.pop((b, hc))
        for ho in range(2):
            h = 2 * hc + ho
            av = ps_av.tile([64, 512], F32, tag="av", name="av_ps")
            for kc in range(2):
                nc.tensor.matmul(av[:, 0:256],
                                 vp_sb[:, b * 2 + kc, h * 64:(h + 1) * 64],
                                 at[ho][:, kc],
                                 start=(kc == 0), stop=(kc == 1))
            for kc in range(2):
                nc.tensor.matmul(av[:, 256:512], ones_sb[:, 0:64],
                                 at[ho][:, kc],
                                 start=(kc == 0), stop=(kc == 1))
            inv = attnp.tile([64, S], F32, tag="inv", name="inv_sb")
            nc.vector.reciprocal_approx_fast(inv[:], av[:, 256:512])
            nc.vector.tensor_tensor(
                out_sb[ho * 64:(ho + 1) * 64, hc, b * S:(b + 1) * S],
                av[:, 0:256], inv[:], OP.mult)

    # ---------------- phase A: V1, Q1, K1 (dense bf16 GEMMs) -----------------
    p_vp = pool("p_vp")
    vp = p_vp.tile([128, 2 * BPC, NH * 64], BF16)
    p_encB = pool("p_encB", bufs=2)
    vnat("wv1", x1T, vp, "v1")
    # enc quarters 2,3 (needed by eva half 1 inside the attn1 window)
    for tq in range(2, 4):
        e_t = p_encB.tile([128, EC, 256], F8, tag="ench", name="ench")
        nc.scalar.dma_start(e_t[:], d["encT"][tq])
        ench.append(e_t)

    p_qt = pool("p_qt"); qt = p_qt.tile([128, HC, T], BF16)
    p_kt = pool("p_kt"); kt = p_kt.tile([128, HC, T], BF16)
    proj_T("q1", "wq1", x1T, HC, copy_act(qt))
    proj_T("k1", "wk1", x1T, HC, copy_act(kt))
    free("p_x1")

    # ---------------- phase B: attn1 || eva th1 || mod quads 16..47 ----------
    p_ao = pool("p_ao", side="right")
    attnout = p_ao.tile([128, HC, T], BF16)
    attnp1 = pool("attnp_attn1", bufs=3, side="right")

    def mod_thunk(o):
        def run():
            with nc.named_scope("mod"):
                mod_quad(o, eng=nc.scalar)
        return run

    def mod_final():
        with nc.named_scope("mod"):
            nc.vector.tensor_tensor(modT[:, 16:48], modT[:, 16:48],
                                    ada_bx4[:, 16:48], OP.add)
            for c in range(HC):
                nc.vector.tensor_scalar(scale3[:, c], modT[:, 32 + c], 1.0,
                                        n_sb["n3T"][:, c:c + 1],
                                        OP.add, OP.mult)

    # filler queue: 2 eva chunks then 1 mod quad, repeating (24 items for the
    # 24 filler slots among 32 units -- every 4th unit runs bare)
    fillers = []
    mq = [mod_thunk(o) for o in range(16, 48, 4)]
    for g in range(8):
        fillers.append(eva_q.pop(0))
        fillers.append(eva_q.pop(0))
        fillers.append(mq.pop(0))

    with nc.named_scope("attn1"):
        # batch-pair interleave: the partner unit's score MMs + fillers hide
        # each unit's exp latency
        for bp in (0, 2):
            for hc in range(NH // 2):
                attn_A(attnp1, qt, kt, bp, hc)
                attn_A(attnp1, qt, kt, bp + 1, hc)
                if fillers:
                    fillers.pop(0)()
                if hc % 2 == 0 and fillers:
                    fillers.pop(0)()
                attn_B(attnp1, vp, attnout, bp, hc)
                attn_B(attnp1, vp, attnout, bp + 1, hc)
        while fillers:
            fillers.pop(0)()
        mod_final()
    free("attnp_attn1")
    free("p_kt"); free("p_qt"); free("p_encB"); free("p_vp")
    free("p_wq4")
    free("p_weva")

    # ---------------- phase C: o1 (+rms2), rms2 tail -------------------------
    p_r2 = pool("p_r2")
    rms2T = p_r2.tile([128, HC, T], F8)
    r2_issue, _, r2_half, r2_release = rms_begin("r2", hsT, side="right")

    # prefetch cross-attention weights while o1 runs
    wv2 = pool("wv_v2", bufs=2)
    wv2_t = []
    for oh in range(2):
        wt = wv2.tile([128, HC, 512], F8, tag="wvnat", name="v2_w")
        nc.sync.dma_start(wt[:], d["wv2"][oh])
        wv2_t.append(wt)
    kq_tiles = []

    def kq_prefetch(hc):
        wtk = wpool.tile([128, HC, 128], F8, tag="w8f", name="k2_w")
        nc.sync.dma_start(wtk[:], d["wk2"][hc])
        wtq = wpool.tile([128, HC, 128], F8, tag="w8f", name="q2_w")
        nc.sync.dma_start(wtq[:], d["wq2"][hc])
        kq_tiles.append((wtk, wtq))

    kq_prefetch(0)

    def resid_gated(g_split, rms_issue=None):
        def c(o, t, p):
            tg = sml.tile([128, 512], F32, tag="resid", name="resid_t")
            for b2 in range(2):
                b = t * 2 + b2
                nc.vector.tensor_scalar(tg[:, b2 * S:(b2 + 1) * S],
                                        p[:, b2 * S:(b2 + 1) * S],
                                        modT[:, g_split * 8 + o, b:b + 1],
                                        None, OP.mult)
            nc.vector.tensor_tensor(hsT[:, o, t * 512:(t + 1) * 512],
                                    hsT[:, o, t * 512:(t + 1) * 512],
                                    tg[:], OP.add)
            if t == 1 and rms_issue is not None:
                rms_issue(o)
        return c

    proj_T("o1", "wo1", attnout, HC, resid_gated(2, r2_issue))

    def rms2_consumer(c, t, xn):
        nc.vector.tensor_scalar(rms2T[:, c, t * 512:(t + 1) * 512], xn[:],
                                n_sb["n2T"][:, c:c + 1], None, OP.mult)

    with nc.named_scope("rms2"):
        r2_half(0, rms2_consumer)   # hsT now holds h1
        r2_half(1, rms2_consumer)
    r2_release()
    free("p_ao")

    # ---------------- phase D: cross attention || k2/q2/v2 (fp8) -------------
    p_ao2 = pool("p_ao2", side="right")
    attn2out = p_ao2.tile([128, HC, T], F8)
    attnp2 = pool("attnp_attn2", bufs=3, side="right")
    p_k2 = pool("p_k2", side="right"); k2t = p_k2.tile([128, HC, T], BF16)
    p_q2 = pool("p_q2", side="right"); q2t = p_q2.tile([128, HC, T], BF16)
    p_v2 = pool("p_v2", side="right")
    vp2 = p_v2.tile([128, 2 * BPC, NH * 64], BF16)

    with nc.named_scope("attn2"):
        # V for b0+b1 (token chunks 0..3, both halves) up front
        for t in range(4):
            for oh in range(2):
                vnat_chunk8("v2", wv2_t[oh], evaT, oh, t, vp2, 1.0 / WS2)
        # b0/b1 unit pairs pipelined with k2/q2 chunk emission; V chunks for
        # b2/b3 ride along as extra filler
        vfill = [(t, oh) for t in range(4, 2 * BPC) for oh in range(2)]
        for hc in range(HC):
            if hc + 1 < HC:
                kq_prefetch(hc + 1)
            wtk, wtq = kq_tiles.pop(0)
            proj_chunk8("k2", wtk, evaT, hc, copy_act_s(k2t, 1.0 / WS2))
            proj_chunk8("q2", wtq, rms2T, hc, copy_act_s(q2t, 1.0 / WS2))
            if hc >= 1:
                attn_B(attnp2, vp2, attn2out, 0, hc - 1)
                attn_B(attnp2, vp2, attn2out, 1, hc - 1)
            attn_A(attnp2, q2t, k2t, 0, hc)
            attn_A(attnp2, q2t, k2t, 1, hc)
            if vfill:
                t, oh = vfill.pop(0)
                vnat_chunk8("v2", wv2_t[oh], evaT, oh, t, vp2, 1.0 / WS2)
        attn_B(attnp2, vp2, attn2out, 0, HC - 1)
        attn_B(attnp2, vp2, attn2out, 1, HC - 1)
    free("wv_v2")
    free("p_r2")
    free("p_eva")

    # ---------------- phase E: o2/rms3 by token halves, then MLP -------------
    p_y = pool("p_y")
    yT = p_y.tile([128, HC, T], BF16)
    _, r3_issue_h, r3_half, _ = rms_begin("r3", hsT, side="left")

    wo2p = pool("p_wo2", bufs=1)
    wo2_t = wo2p.tile([128, HC, HC, 128], F8)
    for o in range(HC):
        nc.sync.dma_start(wo2_t[:, o], d["wo2"][o])

    def resid_plain(o, t, p):
        tmp = sml.tile([128, 512], F32, tag="o2s", bufs=2, name="o2s")
        nc.scalar.activation(tmp[:], p[:], AF.Copy, scale=1.0 / WS2)
        nc.gpsimd.tensor_tensor(hsT[:, o, t * 512:(t + 1) * 512],
                                hsT[:, o, t * 512:(t + 1) * 512],
                                tmp[:], OP.add)
        r3_issue_h(o, t)

    def rms3_consumer(c, t, xn):
        for b2 in range(2):
            b = 2 * t + b2
            nc.vector.tensor_scalar(yT[:, c, b * S:(b + 1) * S],
                                    xn[:, b2 * S:(b2 + 1) * S],
                                    scale3[:, c, b:b + 1],
                                    modT[:, 24 + c, b:b + 1],
                                    OP.mult, OP.add)

    with nc.named_scope("attn2"):
        # b2/b3 unit pairs with o2(t0) chunks as filler
        for hc in range(HC):
            attn_A(attnp2, q2t, k2t, 2, hc)
            attn_A(attnp2, q2t, k2t, 3, hc)
            with nc.named_scope("o2"):
                proj_chunk8("o2", wo2_t[:, hc], attn2out, hc, resid_plain,
                            ts=(0,))
            attn_B(attnp2, vp2, attn2out, 2, hc)
            attn_B(attnp2, vp2, attn2out, 3, hc)
    # q2/k2/v2 and the exp tiles are dead once the b3 units are emitted;
    # free (LIFO) before the MLP reserves its 64KB/partition of mlpT slots.
    free("p_v2"); free("p_q2"); free("p_k2"); free("attnp_attn2")

    # rms3 half0 vector tail hides under the o2(t1) PE stream
    with nc.named_scope("rms3"):
        r3_half(0, rms3_consumer)
    with nc.named_scope("o2"):
        for o in range(HC):
            proj_chunk8("o2", wo2_t[:, o], attn2out, o, resid_plain, ts=(1,))
    free("p_ao2")
    with nc.named_scope("rms3"):
        r3_half(1, rms3_consumer)

    # ---------------- MLP: gate/up/down, weights shared across halves --------
    out_r = r3(outT_d)
    p_mlp = pool("p_mlp")
    wmlp = pool("p_wmlp", bufs=5)
    wdn = pool("p_wdown", bufs=2)
    mlpT = [p_mlp.tile([128, IC, 512], BF16, tag="mlp", bufs=2,
                       name=f"mlpT{t}") for t in range(2)]

    def gu_mm(wg, wu, oc, ts):
        for t in ts:
            pg = ps_proj.tile([128, 512], F32, tag="proj", name="g_ps")
            for f in range(HC):
                nc.tensor.matmul(pg[:], wg[:, f],
                                 yT[:, f, t * 512:(t + 1) * 512],
                                 start=(f == 0), stop=(f == HC - 1))
            pu = ps_proj.tile([128, 512], F32, tag="proj", name="u_ps")
            for f in range(HC):
                nc.tensor.matmul(pu[:], wu[:, f],
                                 yT[:, f, t * 512:(t + 1) * 512],
                                 start=(f == 0), stop=(f == HC - 1))
            gs = sml.tile([128, 512], BF16, tag="gsil", name="gsil")
            nc.scalar.activation(gs[:], pg[:], AF.Silu)
            nc.vector.tensor_tensor(mlpT[t][:, oc], gs[:], pu[:], OP.mult)

    def gu_chunk(oc, ts):
        wg = wmlp.tile([128, HC, 128], BF16, tag="w8", name="gate_w_t")
        nc.sync.dma_start(wg[:], d["gate_w"][oc])
        wu = wmlp.tile([128, HC, 128], BF16, tag="w8", name="up_w_t")
        nc.sync.dma_start(wu[:], d["up_w"][oc])
        gu_mm(wg, wu, oc, ts)

    with nc.named_scope("gateup0"):
        for oc in range(4):
            gu_chunk(oc, (0,))
    with nc.named_scope("gateup1"):
        for oc in range(4, IC):
            gu_chunk(oc, (0, 1))
        for oc in range(4):
            gu_chunk(oc, (1,))

    with nc.named_scope("down0"):
        for o in range(HC):
            wt = wdn.tile([128, IC, 128], BF16, tag="wdown", name="down_w_t")
            nc.sync.dma_start(wt[:], d["down_w"][o])
            for t in range(2):
                p = ps_proj.tile([128, 512], F32, tag="proj", name="d_ps")
                for f in range(IC):
                    nc.tensor.matmul(p[:], wt[:, f], mlpT[t][:, f],
                                     start=(f == 0), stop=(f == IC - 1))
                sl = slice(t * 512, (t + 1) * 512)
                ob = sml.tile([128, 512], F32, tag="outb", bufs=2, name="outb")
                for b2 in range(2):
                    b = 2 * t + b2
                    nc.vector.tensor_scalar(ob[:, b2 * S:(b2 + 1) * S],
                                            p[:, b2 * S:(b2 + 1) * S],
                                            modT[:, 40 + o, b:b + 1],
                                            None, OP.mult)
                nc.vector.tensor_tensor(ob[:], ob[:], hsT[:, o, sl], OP.add)
                nc.sync.dma_start(out_r[:, o, sl], ob[:])

    for nm in reversed(list(open_pools)):
        free(nm)
    for p in list(ms_pools.values()):
        p.release()
    ps_av.release(); ps_sc.release(); ps_proj.release()
    tc_cm.__exit__(None, None, None)
    nc.compile()
    return nc


_CACHE = {}


def _get_program():
    if "nc" not in _CACHE:
        _CACHE["nc"] = build_program()
    return _CACHE["nc"]


def kernel(hidden_states, encoder_hidden_states, timestep_emb,
           wq1, wk1, wv1, wo1, wq2, wk2, wv2, wo2,
           eva_w, eva_b, ada_w, ada_b, gate_w, up_w, down_w, n1, n2, n3,
           _trace=False):
    nc = _get_program()
    f32 = lambda a: np.ascontiguousarray(np.asarray(a), dtype=np.float32)

    def packw(w, cols, dtnp, scale=None):
        """[K, N] weight -> tile-major [N//cols, 128, K//128, cols] so each
        output-chunk weight DMA is one contiguous DRAM read."""
        w = np.asarray(w, np.float32)
        if scale is not None:
            w = w * scale
        K, N = w.shape
        p = w.reshape(K // 128, 128, N // cols, cols).transpose(2, 1, 0, 3)
        return np.ascontiguousarray(p).astype(dtnp)

    cxt, sxt, cyt, syt = _rope_tables()
    ropeT = np.ascontiguousarray(np.stack([cxt, sxt, cyt, syt], axis=1))
    colchunks = lambda v, n: np.asarray(v, np.float32).reshape(n, 128).T
    ada_bT = colchunks(ada_b, 48)
    constsF = np.concatenate([
        colchunks(n1, HC), colchunks(n2, HC), colchunks(n3, HC),
        colchunks(eva_b, HC), ada_bT,
        np.full((128, 1), EPS, np.float32),
        np.repeat(ada_bT, 4, axis=1)], axis=1)
    shared = dict(
        wq1=packw(wq1, 128, BF16NP), wk1=packw(wk1, 128, BF16NP),
        wv1=packw(wv1, 512, BF16NP), wo1=packw(wo1, 128, BF16NP),
        wq2=packw(wq2, 128, F8NP, WS2), wk2=packw(wk2, 128, F8NP, WS2),
        wv2=packw(wv2, 512, F8NP, WS2), wo2=packw(wo2, 128, F8NP, WS2),
        eva_w=packw(eva_w, 128, F8NP, WSE),
        ada_w=packw(ada_w, 512, BF16NP),
        gate_w=packw(gate_w, 128, BF16NP), up_w=packw(up_w, 128, BF16NP),
        down_w=packw(down_w, 128, BF16NP),
        constsF=np.ascontiguousarray(constsF),
        ropeT=ropeT,
        ones=np.ones((128, 128), BF16NP),
    )
    hs = f32(hidden_states)
    enc = f32(encoder_hidden_states)
    temb = f32(timestep_emb)

    in_maps = []
    for c in range(NC_):
        sl = slice(c * BPC, (c + 1) * BPC)
        m = dict(shared)
        m["hsT"] = np.ascontiguousarray(hs[sl].transpose(2, 0, 1).reshape(H, T))
        m["encT"] = packw(enc[sl].transpose(2, 0, 1).reshape(E, T), 256, F8NP)
        m["tembT"] = np.ascontiguousarray(temb[sl].T)
        in_maps.append(m)

    res = run_bass_kernel_spmd(nc, in_maps, core_ids=list(range(NC_)),
                               trace=_trace)
    out = np.empty((B, S, H), np.float32)
    for c in range(NC_):
        o = res.results[c]["outT"]  # [H, T]
        out[c * BPC:(c + 1) * BPC] = np.ascontiguousarray(o.T).reshape(BPC, S, H)
    if _trace:
        kernel.last_results = res
    return out
